# revision 21
# baseline (speedup 1.0000x reference)
"""AMS loss kernel for Trainium2, data-parallel over 8 NeuronCores.

Reference computation (per row r of logits [N, C], target t_r):
    num_r   = logits[r, t_r]
    denom_r = exp(num_r) + (sum_j exp(logits[r, j])) * e^M - exp(num_r) * e^M
    L_r     = num_r - log(denom_r + EPS)
    out     = -mean_r(L_r)

Sharding: rows (N=16384) split evenly across 8 cores (2048 rows each).
Each core streams its shard through SBUF as 16 row-tiles x 4 column
chunks of [128, 2500]; the scalar engine computes exp(x + M) with a
fused row-sum (accum_out), the vector engine gathers the target logit
via a fused (iota == target) * logits multiply-accumulate, and the
epilogue folds everything into one partial sum per core which the host
combines.

Raw Bass (no Tile): the fused-reduce DVE instruction (S2S2D2_STT) and
the HWDGE DMA instruction have too few embedded sync-wait slots for
Tile's auto-generated dependency waits, so synchronization is explicit
standalone wait_ge instructions per engine.

Schedule notes (from NTFF profiling):
 - The logits stream runs at HBM line rate (~390 GB/s) on the SP HWDGE
   FIFO queue; it is the roofline. Everything else must stay off that
   queue and keep up with its period.
 - Column-chunked streaming (2500 cols) keeps the DVE gather (3.1us)
   ahead of the chunk arrival period (3.3us), so only the final chunk's
   gather is exposed after the stream ends (vs 10.6us for a full tile).
 - The targets DMA + int->float cast and the iota ramp run on gpsimd
   (SWDGE/compute) so the SP queue and the DVE stay clean; the iota is
   generated in 4 column chunks so gather (0,0) can start early.
"""

import sys
import numpy as np

for _p in ("/opt/trn_rl_repo",):
    if _p not in sys.path:
        sys.path.insert(0, _p)

N_TOTAL = 16384
C = 10000
N_CORES = 8
ROWS = N_TOTAL // N_CORES        # 2048 rows per core
P = 128                          # partitions
TILES = ROWS // P                # 16 row-tiles per core
M = 0.4
EPS = 1e-10
NBUF = 3                         # row-tile buffer slots
NCH = 4                          # column chunks per tile
CCOLS = C // NCH                 # 2500
NCHUNKS = TILES * NCH            # 64

PROFILE = False                  # set True (e.g. by test.py) to capture NTFF profile
LAST_RESULT = None               # BassKernelResults of the last run (for profiling)

_CACHE = {}


def _build_nc():
    from contextlib import ExitStack

    import concourse.bass as bass
    import concourse.mybir as mybir

    F32 = mybir.dt.float32
    BF16 = mybir.dt.bfloat16
    I32 = mybir.dt.int32
    Alu = mybir.AluOpType
    Act = mybir.ActivationFunctionType

    EXP_M = float(np.exp(np.float32(M)))

    nc = bass.Bass()
    logits = nc.declare_dram_parameter("logits", [ROWS, C], F32, isOutput=False)
    tgt = nc.declare_dram_parameter("tgt", [P, TILES], I32, isOutput=False)
    out = nc.declare_dram_parameter("out", [1, 1], F32, isOutput=True)

    logits_t = logits.rearrange("(n p) c -> n p c", p=P)

    with ExitStack() as ctx:
        en_ctx = ctx.enter_context
        tb = [
            en_ctx(nc.sbuf_tensor(f"tb{i}", [P, C], F32)) for i in range(NBUF)
        ]
        iota_f = en_ctx(nc.sbuf_tensor([P, C], F32))
        g_dve = en_ctx(nc.sbuf_tensor([P, CCOLS], BF16))  # unused elementwise out
        g_act = en_ctx(nc.sbuf_tensor([P, CCOLS], BF16))  # unused elementwise out
        bias_m = en_ctx(nc.sbuf_tensor([P, 1], F32))
        bias_eps = en_ctx(nc.sbuf_tensor([P, 1], F32))
        tgt_i = en_ctx(nc.sbuf_tensor([P, TILES], I32))
        tgt_f = en_ctx(nc.sbuf_tensor([P, TILES], F32))
        summ_x = en_ctx(nc.sbuf_tensor([P, NCHUNKS], F32))
        num_x = en_ctx(nc.sbuf_tensor([P, NCHUNKS], F32))
        summ = en_ctx(nc.sbuf_tensor([P, TILES], F32))
        num = en_ctx(nc.sbuf_tensor([P, TILES], F32))
        en = en_ctx(nc.sbuf_tensor([P, TILES], F32))
        denom = en_ctx(nc.sbuf_tensor([P, TILES], F32))
        lnd = en_ctx(nc.sbuf_tensor([P, TILES], F32))
        lg = en_ctx(nc.sbuf_tensor([P, TILES], F32))
        partial = en_ctx(nc.sbuf_tensor([P, 1], F32))
        red = en_ctx(nc.sbuf_tensor([1, 1], F32))
        res = en_ctx(nc.sbuf_tensor([1, 1], F32))

        tgt_sem = en_ctx(nc.semaphore("tgt_sem"))
        cs = [
            [en_ctx(nc.semaphore(f"cs{s}_{c}")) for c in range(NCH)]
            for s in range(NBUF)
        ]
        out_sem = en_ctx(nc.semaphore("out_sem"))
        v_sem = en_ctx(nc.semaphore("v_sem"))
        a_sem = en_ctx(nc.semaphore("a_sem"))
        b_sem = en_ctx(nc.semaphore("b_sem"))
        p_sem = en_ctx(nc.semaphore("p_sem"))

        block = en_ctx(nc.Block())

        # Semaphore timelines:
        #  cs[j % NBUF][c] == 16*(j//NBUF + 1)  <=>  chunk (j, c) loaded
        #  tgt_sem == 16                        <=>  targets loaded (SWDGE)
        #  b_sem: DVE bias memsets -> 1
        #  p_sem: iota chunk0 -> 1 ; tgt cast -> 2 ; iota chunks 1..3 -> 3..5 ;
        #         red -> 6
        #  v_sem: gather (j,c) -> 4j+c+1 (64 total) ; num fold -> 65 ;
        #         summ fold -> 66 ; denom -> 67 ; lg -> 68
        #  a_sem: exp (j,c) -> 4j+c+1 (64) ; en -> 65 ; lnd -> 66 ; res -> 67

        V_G = NCHUNKS              # 64
        V_NUM = V_G + 1            # 65
        V_SUM = V_NUM + 1          # 66
        V_DEN = V_SUM + 1          # 67
        V_LG = V_DEN + 1           # 68
        A_E = NCHUNKS              # 64
        A_EN = A_E + 1             # 65
        A_LND = A_EN + 1           # 66
        A_RES = A_LND + 1          # 67

        @block.sync
        def _(sync):
            for j in range(TILES):
                if j >= NBUF:
                    # slot reuse: compute of tile j-NBUF fully done
                    sync.wait_ge(v_sem, NCH * (j - NBUF) + NCH)
                    sync.wait_ge(a_sem, NCH * (j - NBUF) + NCH)
                for c in range(NCH):
                    csl = slice(c * CCOLS, (c + 1) * CCOLS)
                    sync.dma_start(
                        out=tb[j % NBUF][:, csl], in_=logits_t[j][:, csl]
                    ).then_inc(cs[j % NBUF][c], 16)
            sync.wait_ge(a_sem, A_RES)
            sync.dma_start(out=out[:], in_=res[:]).then_inc(out_sem, 16)

        @block.gpsimd
        def _(gpsimd):
            gpsimd.dma_start(out=tgt_i[:], in_=tgt[:]).then_inc(tgt_sem, 16)

            def iota_chunk(c):
                gpsimd.iota(
                    iota_f[:, c * CCOLS : (c + 1) * CCOLS],
                    pattern=[[1, CCOLS]],
                    base=c * CCOLS,
                    channel_multiplier=0,
                    allow_small_or_imprecise_dtypes=True,
                ).then_inc(p_sem, 1)

            iota_chunk(0)
            gpsimd.wait_ge(tgt_sem, 16)
            gpsimd.tensor_copy(tgt_f[:], tgt_i[:]).then_inc(p_sem, 1)
            for c in range(1, NCH):
                iota_chunk(c)
            gpsimd.wait_ge(v_sem, V_LG)
            gpsimd.tensor_reduce(
                red[:], partial[:], axis=mybir.AxisListType.C, op=Alu.add
            ).then_inc(p_sem, 1)

        @block.vector
        def _(vector):
            vector.memset(bias_m[:], M)
            vector.memset(bias_eps[:], EPS).then_inc(b_sem, 1)
            for j in range(TILES):
                for c in range(NCH):
                    k = NCH * j + c
                    if j == 0:
                        # iota chunk c + targets cast ready
                        vector.wait_ge(p_sem, 2 if c == 0 else c + 2)
                    csl = slice(c * CCOLS, (c + 1) * CCOLS)
                    vector.wait_ge(v_sem, k)
                    vector.wait_ge(cs[j % NBUF][c], 16 * (j // NBUF + 1))
                    vector.scalar_tensor_tensor(
                        out=g_dve[:],
                        in0=iota_f[:, csl],
                        scalar=tgt_f[:, j : j + 1],
                        in1=tb[j % NBUF][:, csl],
                        op0=Alu.is_equal,
                        op1=Alu.mult,
                        accum_out=num_x[:, k : k + 1],
                    ).then_inc(v_sem, 1)
            # fold chunk partials: [128, 16, 4] -reduce-> [128, 16]
            num_x3 = num_x[:].rearrange("p (t c) -> p t c", c=NCH)
            summ_x3 = summ_x[:].rearrange("p (t c) -> p t c", c=NCH)
            vector.wait_ge(v_sem, V_G)
            vector.tensor_reduce(
                num[:], num_x3, axis=mybir.AxisListType.X, op=Alu.add
            ).then_inc(v_sem, 1)
            vector.wait_ge(a_sem, A_E)
            vector.wait_ge(v_sem, V_NUM)
            vector.tensor_reduce(
                summ[:], summ_x3, axis=mybir.AxisListType.X, op=Alu.add
            ).then_inc(v_sem, 1)
            vector.wait_ge(a_sem, A_EN)
            vector.wait_ge(v_sem, V_SUM)
            vector.scalar_tensor_tensor(
                out=denom[:],
                in0=en[:],
                scalar=1.0 - EXP_M,
                in1=summ[:],
                op0=Alu.mult,
                op1=Alu.add,
            ).then_inc(v_sem, 1)
            vector.wait_ge(a_sem, A_LND)
            vector.wait_ge(v_sem, V_DEN)
            vector.scalar_tensor_tensor(
                out=lg[:],
                in0=num[:],
                scalar=1.0,
                in1=lnd[:],
                op0=Alu.mult,
                op1=Alu.subtract,
                accum_out=partial[:],
            ).then_inc(v_sem, 1)

        @block.scalar
        def _(scalar):
            scalar.wait_ge(b_sem, 1)
            for j in range(TILES):
                for c in range(NCH):
                    k = NCH * j + c
                    csl = slice(c * CCOLS, (c + 1) * CCOLS)
                    scalar.wait_ge(a_sem, k)
                    scalar.wait_ge(cs[j % NBUF][c], 16 * (j // NBUF + 1))
                    scalar.activation(
                        out=g_act[:],
                        in_=tb[j % NBUF][:, csl],
                        func=Act.Exp,
                        bias=bias_m[:],
                        scale=1.0,
                        accum_out=summ_x[:, k : k + 1],
                    ).then_inc(a_sem, 1)
            scalar.wait_ge(v_sem, V_NUM)
            scalar.activation(out=en[:], in_=num[:], func=Act.Exp).then_inc(a_sem, 1)
            scalar.wait_ge(v_sem, V_DEN)
            scalar.activation(
                out=lnd[:], in_=denom[:], func=Act.Ln, bias=bias_eps[:]
            ).then_inc(a_sem, 1)
            scalar.wait_ge(p_sem, NCH + 2)
            scalar.mul(res[:], red[:], -1.0 / N_TOTAL).then_inc(a_sem, 1)

    return nc


def _get_nc():
    if "nc" not in _CACHE:
        _CACHE["nc"] = _build_nc()
    return _CACHE["nc"]


def kernel(logits, targets):
    global LAST_RESULT
    from concourse.bass_utils import run_bass_kernel_spmd

    logits = np.ascontiguousarray(np.asarray(logits), dtype=np.float32)
    targets = np.asarray(targets).astype(np.int32)
    assert logits.shape == (N_TOTAL, C), logits.shape
    assert targets.shape == (N_TOTAL,), targets.shape

    in_maps = []
    for k in range(N_CORES):
        lo, hi = k * ROWS, (k + 1) * ROWS
        shard = logits[lo:hi]
        # tile j, partition p holds row j*128+p -> tgt[p, j] = targets[lo + j*128 + p]
        tgt_shard = np.ascontiguousarray(targets[lo:hi].reshape(TILES, P).T)
        in_maps.append({"logits": shard, "tgt": tgt_shard})

    nc = _get_nc()
    result = run_bass_kernel_spmd(
        nc, in_maps, core_ids=list(range(N_CORES)), trace=PROFILE
    )
    LAST_RESULT = result
    total = np.float64(0.0)
    for r in result.results:
        total += np.float64(r["out"].reshape(-1)[0])
    return np.float32(total)


# revision 23
# speedup vs baseline: 1.0327x; 1.0327x over previous
"""AMS loss kernel for Trainium2, data-parallel over 8 NeuronCores.

Reference computation (per row r of logits [N, C], target t_r):
    num_r   = logits[r, t_r]
    denom_r = exp(num_r) + (sum_j exp(logits[r, j])) * e^M - exp(num_r) * e^M
    L_r     = num_r - log(denom_r + EPS)
    out     = -mean_r(L_r)

Sharding: rows (N=16384) split evenly across 8 cores (2048 rows each).
Each core streams its shard through SBUF in 16 row-tiles of [128, 10000];
the scalar engine computes exp(x + M) with a fused row-sum (accum_out),
the vector engine gathers the target logit via a fused
(iota == target) * logits multiply-accumulate, and the epilogue folds
everything into one partial sum per core which the host combines.

Raw Bass (no Tile): the fused-reduce DVE instruction (S2S2D2_STT) and
the HWDGE DMA instruction have too few embedded sync-wait slots for
Tile's auto-generated dependency waits, so synchronization is explicit
standalone wait_ge instructions per engine.

Schedule notes (from NTFF profiling):
 - The logits stream runs at HBM line rate (~390 GB/s) on the SP HWDGE
   FIFO queue with 40 KB per-partition lines; smaller lines measurably
   drop the rate (10 KB lines -> 344 GB/s), so only the last two tiles
   are column-split (tile 14 in 2, tile 15 in 4) to shrink the
   end-of-stream gather exposure from ~10.6us to ~3us.
 - The targets DMA + int->float cast and the iota ramp run on gpsimd
   (SWDGE/compute) so the SP queue and the DVE stay clean.
"""

import sys
import numpy as np

for _p in ("/opt/trn_rl_repo",):
    if _p not in sys.path:
        sys.path.insert(0, _p)

N_TOTAL = 16384
C = 10000
N_CORES = 8
ROWS = N_TOTAL // N_CORES        # 2048 rows per core
P = 128                          # partitions
TILES = ROWS // P                # 16 row-tiles per core
M = 0.4
EPS = 1e-10
NBUF = 3                         # row-tile buffer slots

# chunks per tile: full tiles keep the DMA at line rate; the last two
# tiles are split so the final exposed gather is small
CHN = [1] * (TILES - 2) + [2, 4]

PROFILE = False                  # set True (e.g. by test.py) to capture NTFF profile
LAST_RESULT = None               # BassKernelResults of the last run (for profiling)

_CACHE = {}


def _build_nc():
    from contextlib import ExitStack

    import concourse.bass as bass
    import concourse.mybir as mybir

    F32 = mybir.dt.float32
    BF16 = mybir.dt.bfloat16
    I32 = mybir.dt.int32
    Alu = mybir.AluOpType
    Act = mybir.ActivationFunctionType

    EXP_M = float(np.exp(np.float32(M)))

    # chunk table: (tile j, chunk c, col_lo, col_hi, overall index k)
    chunks = []
    for j in range(TILES):
        w = C // CHN[j]
        for c in range(CHN[j]):
            chunks.append((j, c, c * w, (c + 1) * w))
    K = len(chunks)                                   # 20
    cum = [0] * (TILES + 1)
    for j in range(TILES):
        cum[j + 1] = cum[j] + CHN[j]

    # multi-chunk tiles accumulate into scratch columns, folded at the end
    xcol = {}                                         # (j, c) -> scratch col
    nx = 0
    for j in range(TILES):
        if CHN[j] > 1:
            for c in range(CHN[j]):
                xcol[(j, c)] = nx
                nx += 1

    # sems per (slot, chunk index): number of chunks ever loaded into that
    # slot position, so each sem's updates are handshake-serialized
    slot_chunks = [0] * NBUF
    for j in range(TILES):
        slot_chunks[j % NBUF] = max(slot_chunks[j % NBUF], CHN[j])
    # per-sem use counters to compute cumulative wait thresholds
    use_count = {}

    V_G = K                      # all gathers done
    N_FOLD = sum(1 for j in range(TILES) if CHN[j] > 1)
    V_NUM = V_G + N_FOLD         # num folds done
    V_SUM = V_NUM + N_FOLD       # summ folds done
    V_DEN = V_SUM + 1
    V_LG = V_DEN + 1
    A_E = K
    A_EN = A_E + 1
    A_LND = A_EN + 1
    A_RES = A_LND + 1

    nc = bass.Bass()
    logits = nc.declare_dram_parameter("logits", [ROWS, C], F32, isOutput=False)
    tgt = nc.declare_dram_parameter("tgt", [P, TILES], I32, isOutput=False)
    out = nc.declare_dram_parameter("out", [1, 1], F32, isOutput=True)

    logits_t = logits.rearrange("(n p) c -> n p c", p=P)

    with ExitStack() as ctx:
        en_ctx = ctx.enter_context
        tb = [
            en_ctx(nc.sbuf_tensor(f"tb{i}", [P, C], F32)) for i in range(NBUF)
        ]
        iota_f = en_ctx(nc.sbuf_tensor([P, C], F32))
        g_dve = en_ctx(nc.sbuf_tensor([P, C], BF16))   # unused elementwise out
        g_act = en_ctx(nc.sbuf_tensor([P, C], BF16))   # unused elementwise out
        bias_m = en_ctx(nc.sbuf_tensor([P, 1], F32))
        bias_eps = en_ctx(nc.sbuf_tensor([P, 1], F32))
        tgt_i = en_ctx(nc.sbuf_tensor([P, TILES], I32))
        tgt_f = en_ctx(nc.sbuf_tensor([P, TILES], F32))
        summ = en_ctx(nc.sbuf_tensor([P, TILES], F32))
        num = en_ctx(nc.sbuf_tensor([P, TILES], F32))
        num_x = en_ctx(nc.sbuf_tensor([P, max(nx, 1)], F32))
        summ_x = en_ctx(nc.sbuf_tensor([P, max(nx, 1)], F32))
        en = en_ctx(nc.sbuf_tensor([P, TILES], F32))
        denom = en_ctx(nc.sbuf_tensor([P, TILES], F32))
        lnd = en_ctx(nc.sbuf_tensor([P, TILES], F32))
        lg = en_ctx(nc.sbuf_tensor([P, TILES], F32))
        partial = en_ctx(nc.sbuf_tensor([P, 1], F32))
        red = en_ctx(nc.sbuf_tensor([1, 1], F32))
        res = en_ctx(nc.sbuf_tensor([1, 1], F32))

        tgt_sem = en_ctx(nc.semaphore("tgt_sem"))
        cs = [
            [en_ctx(nc.semaphore(f"cs{s}_{c}")) for c in range(slot_chunks[s])]
            for s in range(NBUF)
        ]
        out_sem = en_ctx(nc.semaphore("out_sem"))
        v_sem = en_ctx(nc.semaphore("v_sem"))
        a_sem = en_ctx(nc.semaphore("a_sem"))
        b_sem = en_ctx(nc.semaphore("b_sem"))
        p_sem = en_ctx(nc.semaphore("p_sem"))

        block = en_ctx(nc.Block())

        def chunk_wait_threshold(j, c):
            """Cumulative wait value on cs[j % NBUF][c] once chunk (j,c) is
            loaded; call exactly once per consumer stream per chunk."""
            key = ("w", j, c)
            if key not in use_count:
                n = sum(1 for j2 in range(j + 1) if j2 % NBUF == j % NBUF
                        and CHN[j2] > c)
                use_count[key] = 16 * n
            return use_count[key]

        @block.sync
        def _(sync):
            for j, c, lo, hi in chunks:
                if c == 0 and j >= NBUF:
                    # slot reuse: compute of tile j-NBUF fully done
                    sync.wait_ge(v_sem, cum[j - NBUF + 1])
                    sync.wait_ge(a_sem, cum[j - NBUF + 1])
                sync.dma_start(
                    out=tb[j % NBUF][:, lo:hi], in_=logits_t[j][:, lo:hi]
                ).then_inc(cs[j % NBUF][c], 16)
            sync.wait_ge(a_sem, A_RES)
            sync.dma_start(out=out[:], in_=res[:]).then_inc(out_sem, 16)

        @block.gpsimd
        def _(gpsimd):
            gpsimd.dma_start(out=tgt_i[:], in_=tgt[:]).then_inc(tgt_sem, 16)
            gpsimd.iota(
                iota_f[:],
                pattern=[[1, C]],
                base=0,
                channel_multiplier=0,
                allow_small_or_imprecise_dtypes=True,
            ).then_inc(p_sem, 1)
            gpsimd.wait_ge(tgt_sem, 16)
            gpsimd.tensor_copy(tgt_f[:], tgt_i[:]).then_inc(p_sem, 1)
            gpsimd.wait_ge(v_sem, V_LG)
            gpsimd.tensor_reduce(
                red[:], partial[:], axis=mybir.AxisListType.C, op=Alu.add
            ).then_inc(p_sem, 1)

        @block.vector
        def _(vector):
            vector.memset(bias_m[:], M)
            vector.memset(bias_eps[:], EPS).then_inc(b_sem, 1)
            vector.wait_ge(p_sem, 2)   # iota + targets cast
            for k, (j, c, lo, hi) in enumerate(chunks):
                acc = (
                    num[:, j : j + 1]
                    if CHN[j] == 1
                    else num_x[:, xcol[(j, c)] : xcol[(j, c)] + 1]
                )
                vector.wait_ge(v_sem, k)
                vector.wait_ge(cs[j % NBUF][c], chunk_wait_threshold(j, c))
                vector.scalar_tensor_tensor(
                    out=g_dve[:, 0 : hi - lo],
                    in0=iota_f[:, lo:hi],
                    scalar=tgt_f[:, j : j + 1],
                    in1=tb[j % NBUF][:, lo:hi],
                    op0=Alu.is_equal,
                    op1=Alu.mult,
                    accum_out=acc,
                ).then_inc(v_sem, 1)
            # fold multi-chunk tiles' partials into their num/summ column
            v = V_G
            for j in range(TILES):
                if CHN[j] > 1:
                    x0 = xcol[(j, 0)]
                    vector.wait_ge(v_sem, v)
                    vector.tensor_reduce(
                        num[:, j : j + 1],
                        num_x[:, x0 : x0 + CHN[j]],
                        axis=mybir.AxisListType.X,
                        op=Alu.add,
                    ).then_inc(v_sem, 1)
                    v += 1
            vector.wait_ge(a_sem, A_E)
            for j in range(TILES):
                if CHN[j] > 1:
                    x0 = xcol[(j, 0)]
                    vector.wait_ge(v_sem, v)
                    vector.tensor_reduce(
                        summ[:, j : j + 1],
                        summ_x[:, x0 : x0 + CHN[j]],
                        axis=mybir.AxisListType.X,
                        op=Alu.add,
                    ).then_inc(v_sem, 1)
                    v += 1
            vector.wait_ge(a_sem, A_EN)
            vector.wait_ge(v_sem, V_SUM)
            vector.scalar_tensor_tensor(
                out=denom[:],
                in0=en[:],
                scalar=1.0 - EXP_M,
                in1=summ[:],
                op0=Alu.mult,
                op1=Alu.add,
            ).then_inc(v_sem, 1)
            vector.wait_ge(a_sem, A_LND)
            vector.wait_ge(v_sem, V_DEN)
            vector.scalar_tensor_tensor(
                out=lg[:],
                in0=num[:],
                scalar=1.0,
                in1=lnd[:],
                op0=Alu.mult,
                op1=Alu.subtract,
                accum_out=partial[:],
            ).then_inc(v_sem, 1)

        @block.scalar
        def _(scalar):
            scalar.wait_ge(b_sem, 1)
            for k, (j, c, lo, hi) in enumerate(chunks):
                acc = (
                    summ[:, j : j + 1]
                    if CHN[j] == 1
                    else summ_x[:, xcol[(j, c)] : xcol[(j, c)] + 1]
                )
                scalar.wait_ge(a_sem, k)
                scalar.wait_ge(cs[j % NBUF][c], chunk_wait_threshold(j, c))
                scalar.activation(
                    out=g_act[:, 0 : hi - lo],
                    in_=tb[j % NBUF][:, lo:hi],
                    func=Act.Exp,
                    bias=bias_m[:],
                    scale=1.0,
                    accum_out=acc,
                ).then_inc(a_sem, 1)
            scalar.wait_ge(v_sem, V_NUM)
            scalar.activation(out=en[:], in_=num[:], func=Act.Exp).then_inc(a_sem, 1)
            scalar.wait_ge(v_sem, V_DEN)
            scalar.activation(
                out=lnd[:], in_=denom[:], func=Act.Ln, bias=bias_eps[:]
            ).then_inc(a_sem, 1)
            scalar.wait_ge(p_sem, 3)
            scalar.mul(res[:], red[:], -1.0 / N_TOTAL).then_inc(a_sem, 1)

    return nc


def _get_nc():
    if "nc" not in _CACHE:
        _CACHE["nc"] = _build_nc()
    return _CACHE["nc"]


def kernel(logits, targets):
    global LAST_RESULT
    from concourse.bass_utils import run_bass_kernel_spmd

    logits = np.ascontiguousarray(np.asarray(logits), dtype=np.float32)
    targets = np.asarray(targets).astype(np.int32)
    assert logits.shape == (N_TOTAL, C), logits.shape
    assert targets.shape == (N_TOTAL,), targets.shape

    in_maps = []
    for k in range(N_CORES):
        lo, hi = k * ROWS, (k + 1) * ROWS
        shard = logits[lo:hi]
        # tile j, partition p holds row j*128+p -> tgt[p, j] = targets[lo + j*128 + p]
        tgt_shard = np.ascontiguousarray(targets[lo:hi].reshape(TILES, P).T)
        in_maps.append({"logits": shard, "tgt": tgt_shard})

    nc = _get_nc()
    result = run_bass_kernel_spmd(
        nc, in_maps, core_ids=list(range(N_CORES)), trace=PROFILE
    )
    LAST_RESULT = result
    total = np.float64(0.0)
    for r in result.results:
        total += np.float64(r["out"].reshape(-1)[0])
    return np.float32(total)


# revision 30
# speedup vs baseline: 1.0656x; 1.0319x over previous
"""AMS loss kernel for Trainium2, data-parallel over 8 NeuronCores.

Reference computation (per row r of logits [N, C], target t_r):
    num_r   = logits[r, t_r]
    denom_r = exp(num_r) + (sum_j exp(logits[r, j])) * e^M - exp(num_r) * e^M
    L_r     = num_r - log(denom_r + EPS)
    out     = -mean_r(L_r)

Sharding: rows (N=16384) split evenly across 8 cores (2048 rows each).
Each core streams its shard through SBUF in 16 row-tiles of [128, 10000];
the scalar engine computes exp(x + M) with a fused row-sum (accum_out),
the vector engine (helped by gpsimd) gathers the target logit via a
fused (iota == target) * logits multiply-accumulate, and the epilogue
folds everything into one partial sum per core which the host combines.

Raw Bass (no Tile): the fused-reduce DVE instruction (S2S2D2_STT) and
the HWDGE DMA instruction have too few embedded sync-wait slots for
Tile's auto-generated dependency waits, so synchronization is explicit
standalone wait_ge instructions per engine.

Schedule notes (from NTFF profiling):
 - The logits stream runs at HBM line rate (~390-398 GB/s) on the SP
   HWDGE FIFO queue with 40 KB per-partition lines; smaller lines drop
   the rate (10 KB -> ~380, 4 KB -> ~344), so only the last two tiles
   are column-split (4 x 2500 each) to shrink end-of-stream exposure.
 - The chip power-shares between clock domains run-to-run: some runs
   have ~20% slower compute clocks. gpsimd takes 5 of the 16 gathers so
   the vector engine keeps slack in both regimes.
 - The targets DMA + int->float cast and the iota ramp run on gpsimd
   so the SP queue and the DVE stay clean.
"""

import sys
import numpy as np

for _p in ("/opt/trn_rl_repo",):
    if _p not in sys.path:
        sys.path.insert(0, _p)

N_TOTAL = 16384
C = 10000
N_CORES = 8
ROWS = N_TOTAL // N_CORES        # 2048 rows per core
P = 128                          # partitions
TILES = ROWS // P                # 16 row-tiles per core
M = 0.4
EPS = 1e-10
NBUF = 3                         # row-tile buffer slots

# chunks per tile (full tiles keep the DMA at line rate; the last two are
# split so the final exposed gathers are small)
CHN = [4] + [1] * (TILES - 3) + [2, 4]
# tiles whose gather runs on gpsimd instead of the vector engine
GP_TILES = ()

PROFILE = False                  # set True (e.g. by test.py) to capture NTFF profile
LAST_RESULT = None               # BassKernelResults of the last run (for profiling)

_CACHE = {}


def _build_nc():
    from contextlib import ExitStack

    import concourse.bass as bass
    import concourse.mybir as mybir

    F32 = mybir.dt.float32
    BF16 = mybir.dt.bfloat16
    FP8 = mybir.dt.float8e4
    I32 = mybir.dt.int32
    Alu = mybir.AluOpType
    Act = mybir.ActivationFunctionType

    EXP_M = float(np.exp(np.float32(M)))

    assert all(CHN[j] == 1 for j in GP_TILES)

    # chunk table: (tile j, chunk c, col_lo, col_hi)
    chunks = []
    for j in range(TILES):
        w = C // CHN[j]
        for c in range(CHN[j]):
            chunks.append((j, c, c * w, (c + 1) * w))
    K = len(chunks)

    # cumulative gather counts per engine, and exp counts, through tile j
    vcum = [0] * (TILES + 1)     # DVE gathers
    gcum = [0] * (TILES + 1)     # gpsimd gathers
    acum = [0] * (TILES + 1)     # ACT exps (all chunks)
    for j in range(TILES):
        gp = j in GP_TILES
        vcum[j + 1] = vcum[j] + (0 if gp else CHN[j])
        gcum[j + 1] = gcum[j] + (1 if gp else 0)
        acum[j + 1] = acum[j] + CHN[j]
    NG = gcum[TILES]             # gpsimd gathers (5)

    # multi-chunk tiles accumulate into scratch columns, folded at the end
    xcol = {}
    nx = 0
    for j in range(TILES):
        if CHN[j] > 1:
            for c in range(CHN[j]):
                xcol[(j, c)] = nx
                nx += 1
    N_FOLD = sum(1 for j in range(TILES) if CHN[j] > 1)

    V_G = vcum[TILES]            # all DVE gathers done
    V_NUM = V_G + N_FOLD
    V_SUM = V_NUM + N_FOLD
    V_DEN = V_SUM + 1
    V_LG = V_DEN + 1
    A_E = acum[TILES]
    A_MRG = A_E + (1 if NG else 0)   # gpsimd num columns merged (if any)
    A_EN = A_MRG + 1
    A_LND = A_EN + 1
    A_RES = A_LND + 1

    # sems per (slot, chunk position)
    slot_chunks = [0] * NBUF
    for j in range(TILES):
        slot_chunks[j % NBUF] = max(slot_chunks[j % NBUF], CHN[j])

    nc = bass.Bass()
    logits = nc.declare_dram_parameter("logits", [ROWS, C], F32, isOutput=False)
    tgt = nc.declare_dram_parameter("tgt", [P, TILES], I32, isOutput=False)
    out = nc.declare_dram_parameter("out", [1, 1], F32, isOutput=True)

    logits_t = logits.rearrange("(n p) c -> n p c", p=P)

    with ExitStack() as ctx:
        en_ctx = ctx.enter_context
        tb = [
            en_ctx(nc.sbuf_tensor(f"tb{i}", [P, C], F32)) for i in range(NBUF)
        ]
        iota_f = en_ctx(nc.sbuf_tensor([P, C], F32))
        g_dve = en_ctx(nc.sbuf_tensor([P, C], FP8))   # unused elementwise out
        g_act = en_ctx(nc.sbuf_tensor([P, C], FP8))   # unused elementwise out
        bias_m = en_ctx(nc.sbuf_tensor([P, 1], F32))
        bias_eps = en_ctx(nc.sbuf_tensor([P, 1], F32))
        tgt_i = en_ctx(nc.sbuf_tensor([P, TILES], I32))
        tgt_f = en_ctx(nc.sbuf_tensor([P, TILES], F32))
        summ = en_ctx(nc.sbuf_tensor([P, TILES], F32))
        num = en_ctx(nc.sbuf_tensor([P, TILES], F32))
        num_g = en_ctx(nc.sbuf_tensor([P, max(NG, 1)], F32))
        num_x = en_ctx(nc.sbuf_tensor([P, max(nx, 1)], F32))
        summ_x = en_ctx(nc.sbuf_tensor([P, max(nx, 1)], F32))
        en = en_ctx(nc.sbuf_tensor([P, TILES], F32))
        denom = en_ctx(nc.sbuf_tensor([P, TILES], F32))
        lnd = en_ctx(nc.sbuf_tensor([P, TILES], F32))
        lg = en_ctx(nc.sbuf_tensor([P, TILES], F32))
        partial = en_ctx(nc.sbuf_tensor([P, 1], F32))
        red = en_ctx(nc.sbuf_tensor([1, 1], F32))
        res = en_ctx(nc.sbuf_tensor([1, 1], F32))

        tgt_sem = en_ctx(nc.semaphore("tgt_sem"))
        cs = [
            [en_ctx(nc.semaphore(f"cs{s}_{c}")) for c in range(slot_chunks[s])]
            for s in range(NBUF)
        ]
        out_sem = en_ctx(nc.semaphore("out_sem"))
        v_sem = en_ctx(nc.semaphore("v_sem"))
        g_sem = en_ctx(nc.semaphore("g_sem"))
        a_sem = en_ctx(nc.semaphore("a_sem"))
        b_sem = en_ctx(nc.semaphore("b_sem"))
        p_sem = en_ctx(nc.semaphore("p_sem"))

        block = en_ctx(nc.Block())

        _thr = {}

        def chunk_wait_threshold(j, c):
            """Cumulative value of cs[j % NBUF][c] once chunk (j, c) landed."""
            key = (j, c)
            if key not in _thr:
                n = sum(1 for j2 in range(j + 1) if j2 % NBUF == j % NBUF
                        and CHN[j2] > c)
                _thr[key] = 16 * n
            return _thr[key]

        def wait_tile_compute_done(engine, j):
            """Engine waits until tile j's gather + exp both retired."""
            if j in GP_TILES:
                engine.wait_ge(g_sem, gcum[j + 1])
            else:
                engine.wait_ge(v_sem, vcum[j + 1])
            engine.wait_ge(a_sem, acum[j + 1])

        @block.sync
        def _(sync):
            for j, c, lo, hi in chunks:
                if c == 0 and j >= NBUF:
                    wait_tile_compute_done(sync, j - NBUF)
                sync.dma_start(
                    out=tb[j % NBUF][:, lo:hi], in_=logits_t[j][:, lo:hi]
                ).then_inc(cs[j % NBUF][c], 16)
            sync.wait_ge(a_sem, A_RES)
            sync.dma_start(out=out[:], in_=res[:]).then_inc(out_sem, 16)

        @block.gpsimd
        def _(gpsimd):
            gpsimd.dma_start(out=tgt_i[:], in_=tgt[:]).then_inc(tgt_sem, 16)
            # iota in 4 column chunks so gather(0,0) can start early; the
            # targets cast slots in right after the first chunk
            IW = C // 4
            gpsimd.iota(
                iota_f[:, 0:IW],
                pattern=[[1, IW]],
                base=0,
                channel_multiplier=0,
                allow_small_or_imprecise_dtypes=True,
            ).then_inc(p_sem, 1)
            gpsimd.wait_ge(tgt_sem, 16)
            gpsimd.tensor_copy(tgt_f[:], tgt_i[:]).then_inc(p_sem, 1)
            for ic in range(1, 4):
                gpsimd.iota(
                    iota_f[:, ic * IW : (ic + 1) * IW],
                    pattern=[[1, IW]],
                    base=ic * IW,
                    channel_multiplier=0,
                    allow_small_or_imprecise_dtypes=True,
                ).then_inc(p_sem, 1)
            gpsimd.wait_ge(v_sem, V_LG)
            gpsimd.tensor_reduce(
                red[:], partial[:], axis=mybir.AxisListType.C, op=Alu.add
            ).then_inc(p_sem, 1)

        @block.vector
        def _(vector):
            vector.memset(bias_m[:], M)
            vector.memset(bias_eps[:], EPS).then_inc(b_sem, 1)
            vector.wait_ge(p_sem, 2)   # iota + targets cast
            k = 0
            for j, c, lo, hi in chunks:
                if j in GP_TILES:
                    continue
                acc = (
                    num[:, j : j + 1]
                    if CHN[j] == 1
                    else num_x[:, xcol[(j, c)] : xcol[(j, c)] + 1]
                )
                if j == 0:
                    # iota chunk c + targets cast ready (iota0->1, cast->2,
                    # iota c -> c+2)
                    vector.wait_ge(p_sem, 2 if c == 0 else c + 2)
                vector.wait_ge(v_sem, k)
                vector.wait_ge(cs[j % NBUF][c], chunk_wait_threshold(j, c))
                vector.scalar_tensor_tensor(
                    out=g_dve[:, 0 : hi - lo],
                    in0=iota_f[:, lo:hi],
                    scalar=tgt_f[:, j : j + 1],
                    in1=tb[j % NBUF][:, lo:hi],
                    op0=Alu.is_equal,
                    op1=Alu.mult,
                    accum_out=acc,
                ).then_inc(v_sem, 1)
                k += 1
            # fold multi-chunk tiles' partials into their num/summ column
            v = V_G
            for j in range(TILES):
                if CHN[j] > 1:
                    x0 = xcol[(j, 0)]
                    vector.wait_ge(v_sem, v)
                    vector.tensor_reduce(
                        num[:, j : j + 1],
                        num_x[:, x0 : x0 + CHN[j]],
                        axis=mybir.AxisListType.X,
                        op=Alu.add,
                    ).then_inc(v_sem, 1)
                    v += 1
            vector.wait_ge(a_sem, A_E)
            for j in range(TILES):
                if CHN[j] > 1:
                    x0 = xcol[(j, 0)]
                    vector.wait_ge(v_sem, v)
                    vector.tensor_reduce(
                        summ[:, j : j + 1],
                        summ_x[:, x0 : x0 + CHN[j]],
                        axis=mybir.AxisListType.X,
                        op=Alu.add,
                    ).then_inc(v_sem, 1)
                    v += 1
            # denom = en * (1 - e^M) + summ ; gpsimd tiles' en comes from en_g
            vector.wait_ge(a_sem, A_EN)
            vector.wait_ge(v_sem, V_SUM)
            vector.scalar_tensor_tensor(
                out=denom[:],
                in0=en[:],
                scalar=1.0 - EXP_M,
                in1=summ[:],
                op0=Alu.mult,
                op1=Alu.add,
            ).then_inc(v_sem, 1)
            vector.wait_ge(a_sem, A_LND)
            vector.wait_ge(v_sem, V_DEN)
            vector.scalar_tensor_tensor(
                out=lg[:],
                in0=num[:],
                scalar=1.0,
                in1=lnd[:],
                op0=Alu.mult,
                op1=Alu.subtract,
                accum_out=partial[:],
            ).then_inc(v_sem, 1)

        @block.scalar
        def _(scalar):
            scalar.wait_ge(b_sem, 1)
            k = 0
            for j, c, lo, hi in chunks:
                acc = (
                    summ[:, j : j + 1]
                    if CHN[j] == 1
                    else summ_x[:, xcol[(j, c)] : xcol[(j, c)] + 1]
                )
                scalar.wait_ge(a_sem, k)
                scalar.wait_ge(cs[j % NBUF][c], chunk_wait_threshold(j, c))
                scalar.activation(
                    out=g_act[:, 0 : hi - lo],
                    in_=tb[j % NBUF][:, lo:hi],
                    func=Act.Exp,
                    bias=bias_m[:],
                    scale=1.0,
                    accum_out=acc,
                ).then_inc(a_sem, 1)
                k += 1
            # merge gpsimd tiles' gathered columns into num, then en = exp(num);
            # the single inc covers the copies too (in-order retirement)
            scalar.wait_ge(v_sem, V_NUM)
            if NG:
                scalar.wait_ge(g_sem, NG)
                gp_sorted = sorted(GP_TILES)
                for i, j in enumerate(gp_sorted):
                    op = scalar.copy(num[:, j : j + 1], num_g[:, i : i + 1])
                    if j == gp_sorted[-1]:
                        op.then_inc(a_sem, 1)
                scalar.wait_ge(a_sem, A_MRG)
            scalar.activation(out=en[:], in_=num[:], func=Act.Exp).then_inc(
                a_sem, 1
            )
            scalar.wait_ge(v_sem, V_DEN)
            scalar.activation(
                out=lnd[:], in_=denom[:], func=Act.Ln, bias=bias_eps[:]
            ).then_inc(a_sem, 1)
            scalar.wait_ge(p_sem, 6)
            scalar.mul(res[:], red[:], -1.0 / N_TOTAL).then_inc(a_sem, 1)

    return nc


def _get_nc():
    if "nc" not in _CACHE:
        _CACHE["nc"] = _build_nc()
    return _CACHE["nc"]


def kernel(logits, targets):
    global LAST_RESULT
    from concourse.bass_utils import run_bass_kernel_spmd

    logits = np.ascontiguousarray(np.asarray(logits), dtype=np.float32)
    targets = np.asarray(targets).astype(np.int32)
    assert logits.shape == (N_TOTAL, C), logits.shape
    assert targets.shape == (N_TOTAL,), targets.shape

    in_maps = []
    for k in range(N_CORES):
        lo, hi = k * ROWS, (k + 1) * ROWS
        shard = logits[lo:hi]
        # tile j, partition p holds row j*128+p -> tgt[p, j] = targets[lo + j*128 + p]
        tgt_shard = np.ascontiguousarray(targets[lo:hi].reshape(TILES, P).T)
        in_maps.append({"logits": shard, "tgt": tgt_shard})

    nc = _get_nc()
    result = run_bass_kernel_spmd(
        nc, in_maps, core_ids=list(range(N_CORES)), trace=PROFILE
    )
    LAST_RESULT = result
    total = np.float64(0.0)
    for r in result.results:
        total += np.float64(r["out"].reshape(-1)[0])
    return np.float32(total)


# revision 31
# speedup vs baseline: 1.1398x; 1.0696x over previous
"""AMS loss kernel for Trainium2, data-parallel over 8 NeuronCores.

Reference computation (per row r of logits [N, C], target t_r):
    num_r   = logits[r, t_r]
    denom_r = exp(num_r) + (sum_j exp(logits[r, j])) * e^M - exp(num_r) * e^M
    L_r     = num_r - log(denom_r + EPS)
    out     = -mean_r(L_r)

Sharding: rows (N=16384) split evenly across 8 cores (2048 rows each).
Per core:
 - The target logits num_r are fetched straight from DRAM by one
   indirect (gathering) DMA on gpsimd's software DGE, using host-packed
   flat element offsets -- no compute-engine gather pass at all.
 - The scalar engine streams the shard (16 row-tiles of [128, 10000])
   computing exp(x + M) with a fused per-row accumulate (accum_out).
 - The vector engine computes the tiny [128, 16] epilogue; gpsimd folds
   the per-row losses across partitions; the host sums 8 partial scalars.

Raw Bass (no Tile): Tile's auto-generated per-instruction waits overflow
the small sync-wait slot budgets of the fused-reduce and DMA instruction
formats, so synchronization is explicit standalone wait_ge per engine.

Schedule notes (from NTFF profiling):
 - The logits stream runs at HBM line rate (~390-412 GB/s) on the SP
   HWDGE FIFO queue with 40 KB per-partition lines; smaller lines drop
   the rate, so only the last tile is column-split (4 x 2500) to shrink
   the end-of-stream exposure to one small exp.
 - The chip power-shares between clock domains run-to-run (some runs
   have ~20% slower compute clocks, some ~15% slower HBM); with the
   gather off the vector engine, the scalar engine's exp is the only
   per-tile compute and it has slack in both regimes.
"""

import sys
import numpy as np

for _p in ("/opt/trn_rl_repo",):
    if _p not in sys.path:
        sys.path.insert(0, _p)

N_TOTAL = 16384
C = 10000
N_CORES = 8
ROWS = N_TOTAL // N_CORES        # 2048 rows per core
P = 128                          # partitions
TILES = ROWS // P                # 16 row-tiles per core
M = 0.4
EPS = 1e-10
NBUF = 4                         # row-tile buffer slots

# chunks per tile: the last tile is split so the final exposed exp is small
CHN = [1] * (TILES - 1) + [4]

PROFILE = False                  # set True (e.g. by test.py) to capture NTFF profile
LAST_RESULT = None               # BassKernelResults of the last run (for profiling)

_CACHE = {}


def _build_nc():
    from contextlib import ExitStack

    import concourse.bass as bass
    import concourse.mybir as mybir

    F32 = mybir.dt.float32
    FP8 = mybir.dt.float8e4
    I32 = mybir.dt.int32
    Alu = mybir.AluOpType
    Act = mybir.ActivationFunctionType

    EXP_M = float(np.exp(np.float32(M)))

    # chunk table: (tile j, chunk c, col_lo, col_hi)
    chunks = []
    for j in range(TILES):
        w = C // CHN[j]
        for c in range(CHN[j]):
            chunks.append((j, c, c * w, (c + 1) * w))

    acum = [0] * (TILES + 1)     # cumulative exp count through tile j
    for j in range(TILES):
        acum[j + 1] = acum[j] + CHN[j]

    # multi-chunk tiles accumulate into scratch columns, folded at the end
    xcol = {}
    nx = 0
    for j in range(TILES):
        if CHN[j] > 1:
            for c in range(CHN[j]):
                xcol[(j, c)] = nx
                nx += 1
    N_FOLD = sum(1 for j in range(TILES) if CHN[j] > 1)

    A_E = acum[TILES]            # all exps done
    A_EN = A_E + 1
    A_LND = A_EN + 1
    A_RES = A_LND + 1
    V_FOLD = N_FOLD              # summ folds done
    V_DEN = V_FOLD + 1
    V_LG = V_DEN + 1

    slot_chunks = [0] * NBUF
    for j in range(TILES):
        slot_chunks[j % NBUF] = max(slot_chunks[j % NBUF], CHN[j])

    nc = bass.Bass()
    logits = nc.declare_dram_parameter("logits", [ROWS, C], F32, isOutput=False)
    toff = nc.declare_dram_parameter("toff", [P, TILES], I32, isOutput=False)
    out = nc.declare_dram_parameter("out", [1, 1], F32, isOutput=True)

    logits_t = logits.rearrange("(n p) c -> n p c", p=P)
    logits_flat = logits.rearrange("r c -> (r c) ()")

    with ExitStack() as ctx:
        en_ctx = ctx.enter_context
        tb = [
            en_ctx(nc.sbuf_tensor(f"tb{i}", [P, C], F32)) for i in range(NBUF)
        ]
        g_act = en_ctx(nc.sbuf_tensor([P, C], FP8))   # unused elementwise out
        bias_m = en_ctx(nc.sbuf_tensor([P, 1], F32))
        bias_eps = en_ctx(nc.sbuf_tensor([P, 1], F32))
        off_sb = en_ctx(nc.sbuf_tensor([P, TILES], I32))
        summ = en_ctx(nc.sbuf_tensor([P, TILES], F32))
        summ_x = en_ctx(nc.sbuf_tensor([P, max(nx, 1)], F32))
        num = en_ctx(nc.sbuf_tensor([P, TILES], F32))
        en = en_ctx(nc.sbuf_tensor([P, TILES], F32))
        denom = en_ctx(nc.sbuf_tensor([P, TILES], F32))
        lnd = en_ctx(nc.sbuf_tensor([P, TILES], F32))
        lg = en_ctx(nc.sbuf_tensor([P, TILES], F32))
        partial = en_ctx(nc.sbuf_tensor([P, 1], F32))
        red = en_ctx(nc.sbuf_tensor([1, 1], F32))
        res = en_ctx(nc.sbuf_tensor([1, 1], F32))

        to_sem = en_ctx(nc.semaphore("to_sem"))
        num_sem = en_ctx(nc.semaphore("num_sem"))
        cs = [
            [en_ctx(nc.semaphore(f"cs{s}_{c}")) for c in range(slot_chunks[s])]
            for s in range(NBUF)
        ]
        out_sem = en_ctx(nc.semaphore("out_sem"))
        v_sem = en_ctx(nc.semaphore("v_sem"))
        a_sem = en_ctx(nc.semaphore("a_sem"))
        b_sem = en_ctx(nc.semaphore("b_sem"))
        p_sem = en_ctx(nc.semaphore("p_sem"))

        block = en_ctx(nc.Block())

        _thr = {}

        def chunk_wait_threshold(j, c):
            """Cumulative value of cs[j % NBUF][c] once chunk (j, c) landed."""
            key = (j, c)
            if key not in _thr:
                n = sum(1 for j2 in range(j + 1) if j2 % NBUF == j % NBUF
                        and CHN[j2] > c)
                _thr[key] = 16 * n
            return _thr[key]

        @block.sync
        def _(sync):
            for j, c, lo, hi in chunks:
                if c == 0 and j >= NBUF:
                    # slot reuse: only the scalar engine reads tiles now
                    sync.wait_ge(a_sem, acum[j - NBUF + 1])
                sync.dma_start(
                    out=tb[j % NBUF][:, lo:hi], in_=logits_t[j][:, lo:hi]
                ).then_inc(cs[j % NBUF][c], 16)
            sync.wait_ge(a_sem, A_RES)
            sync.dma_start(out=out[:], in_=res[:]).then_inc(out_sem, 16)

        @block.gpsimd
        def _(gpsimd):
            gpsimd.dma_start(out=off_sb.ap(), in_=toff[:]).then_inc(to_sem, 16)
            gpsimd.wait_ge(to_sem, 16)
            # one gathering DMA fetches every target logit straight from DRAM
            gpsimd.indirect_dma_start(
                out=num.ap(),
                out_offset=None,
                in_=logits_flat,
                in_offset=bass.IndirectOffsetOnAxis(ap=off_sb.ap(), axis=0),
            ).then_inc(num_sem, 16)
            gpsimd.wait_ge(v_sem, V_LG)
            gpsimd.tensor_reduce(
                red[:], partial[:], axis=mybir.AxisListType.C, op=Alu.add
            ).then_inc(p_sem, 1)

        @block.vector
        def _(vector):
            vector.memset(bias_m[:], M)
            vector.memset(bias_eps[:], EPS).then_inc(b_sem, 1)
            # fold multi-chunk tiles' partial sums into their summ column
            vector.wait_ge(a_sem, A_E)
            v = 0
            for j in range(TILES):
                if CHN[j] > 1:
                    x0 = xcol[(j, 0)]
                    vector.wait_ge(v_sem, v)
                    vector.tensor_reduce(
                        summ[:, j : j + 1],
                        summ_x[:, x0 : x0 + CHN[j]],
                        axis=mybir.AxisListType.X,
                        op=Alu.add,
                    ).then_inc(v_sem, 1)
                    v += 1
            # denom = en * (1 - e^M) + summ
            vector.wait_ge(a_sem, A_EN)
            vector.wait_ge(v_sem, V_FOLD)
            vector.scalar_tensor_tensor(
                out=denom[:],
                in0=en[:],
                scalar=1.0 - EXP_M,
                in1=summ[:],
                op0=Alu.mult,
                op1=Alu.add,
            ).then_inc(v_sem, 1)
            # L = num - ln(denom + eps), accumulated per row
            vector.wait_ge(a_sem, A_LND)
            vector.wait_ge(v_sem, V_DEN)
            vector.wait_ge(num_sem, 16)
            vector.scalar_tensor_tensor(
                out=lg[:],
                in0=num[:],
                scalar=1.0,
                in1=lnd[:],
                op0=Alu.mult,
                op1=Alu.subtract,
                accum_out=partial[:],
            ).then_inc(v_sem, 1)

        @block.scalar
        def _(scalar):
            scalar.wait_ge(b_sem, 1)
            k = 0
            for j, c, lo, hi in chunks:
                acc = (
                    summ[:, j : j + 1]
                    if CHN[j] == 1
                    else summ_x[:, xcol[(j, c)] : xcol[(j, c)] + 1]
                )
                scalar.wait_ge(a_sem, k)
                scalar.wait_ge(cs[j % NBUF][c], chunk_wait_threshold(j, c))
                scalar.activation(
                    out=g_act[:, 0 : hi - lo],
                    in_=tb[j % NBUF][:, lo:hi],
                    func=Act.Exp,
                    bias=bias_m[:],
                    scale=1.0,
                    accum_out=acc,
                ).then_inc(a_sem, 1)
                k += 1
            scalar.wait_ge(num_sem, 16)
            scalar.activation(out=en[:], in_=num[:], func=Act.Exp).then_inc(
                a_sem, 1
            )
            scalar.wait_ge(v_sem, V_DEN)
            scalar.activation(
                out=lnd[:], in_=denom[:], func=Act.Ln, bias=bias_eps[:]
            ).then_inc(a_sem, 1)
            scalar.wait_ge(p_sem, 1)
            scalar.mul(res[:], red[:], -1.0 / N_TOTAL).then_inc(a_sem, 1)

    return nc


def _get_nc():
    if "nc" not in _CACHE:
        _CACHE["nc"] = _build_nc()
    return _CACHE["nc"]


def kernel(logits, targets):
    global LAST_RESULT
    from concourse.bass_utils import run_bass_kernel_spmd

    logits = np.ascontiguousarray(np.asarray(logits), dtype=np.float32)
    targets = np.asarray(targets).astype(np.int64)
    assert logits.shape == (N_TOTAL, C), logits.shape
    assert targets.shape == (N_TOTAL,), targets.shape

    # tile j, partition p holds shard row j*128 + p; offsets are flat element
    # indices into the core's [ROWS, C] shard for the indirect gather DMA
    rows = np.arange(TILES)[None, :] * P + np.arange(P)[:, None]   # [P, TILES]

    in_maps = []
    for k in range(N_CORES):
        lo, hi = k * ROWS, (k + 1) * ROWS
        shard = logits[lo:hi]
        tg = targets[lo:hi]
        toff = (rows * C + tg[rows]).astype(np.int32)
        in_maps.append({"logits": shard, "toff": np.ascontiguousarray(toff)})

    nc = _get_nc()
    result = run_bass_kernel_spmd(
        nc, in_maps, core_ids=list(range(N_CORES)), trace=PROFILE
    )
    LAST_RESULT = result
    total = np.float64(0.0)
    for r in result.results:
        total += np.float64(r["out"].reshape(-1)[0])
    return np.float32(total)


# revision 33
# speedup vs baseline: 1.1943x; 1.0479x over previous
"""AMS loss kernel for Trainium2, data-parallel over 8 NeuronCores.

Reference computation (per row r of logits [N, C], target t_r):
    num_r   = logits[r, t_r]
    denom_r = exp(num_r) + (sum_j exp(logits[r, j])) * e^M - exp(num_r) * e^M
    L_r     = num_r - log(denom_r + EPS)
    out     = -mean_r(L_r)

Sharding: rows (N=16384) split evenly across 8 cores (2048 rows each).
Per core:
 - The target logits num_r are fetched straight from DRAM by one
   indirect (gathering) DMA on gpsimd's software DGE, using host-packed
   flat element offsets -- no compute-engine gather pass at all.
 - The scalar engine streams the shard (16 row-tiles of [128, 10000])
   computing exp(x + M) with a fused per-row accumulate (accum_out).
 - The vector engine computes the tiny [128, 16] epilogue; gpsimd folds
   the per-row losses across partitions; the host sums 8 partial scalars.

Raw Bass (no Tile): Tile's auto-generated per-instruction waits overflow
the small sync-wait slot budgets of the fused-reduce and DMA instruction
formats, so synchronization is explicit standalone wait_ge per engine.

Schedule notes (from NTFF profiling):
 - The logits stream runs at HBM line rate (~390-412 GB/s) on the SP
   HWDGE FIFO queue with 40 KB per-partition lines; smaller lines drop
   the rate, so only the last tile is column-split (4 x 2500) to shrink
   the end-of-stream exposure to one small exp.
 - The chip power-shares between clock domains run-to-run (some runs
   have ~20% slower compute clocks, some ~15% slower HBM); with the
   gather off the vector engine, the scalar engine's exp is the only
   per-tile compute and it has slack in both regimes.
"""

import sys
import numpy as np

for _p in ("/opt/trn_rl_repo",):
    if _p not in sys.path:
        sys.path.insert(0, _p)

N_TOTAL = 16384
C = 10000
N_CORES = 8
ROWS = N_TOTAL // N_CORES        # 2048 rows per core
P = 128                          # partitions
TILES = ROWS // P                # 16 row-tiles per core
M = 0.4
EPS = 1e-10
NBUF = 4                         # row-tile buffer slots

# chunks per tile: the last tile is split so the final exposed exp is small
CHN = [1] * (TILES - 1) + [4]

PROFILE = False                  # set True (e.g. by test.py) to capture NTFF profile
LAST_RESULT = None               # BassKernelResults of the last run (for profiling)

_CACHE = {}


def _build_nc():
    from contextlib import ExitStack

    import concourse.bass as bass
    import concourse.mybir as mybir

    F32 = mybir.dt.float32
    FP8 = mybir.dt.float8e4
    I32 = mybir.dt.int32
    Alu = mybir.AluOpType
    Act = mybir.ActivationFunctionType

    EXP_M = float(np.exp(np.float32(M)))

    # chunk table: (tile j, chunk c, col_lo, col_hi)
    chunks = []
    for j in range(TILES):
        w = C // CHN[j]
        for c in range(CHN[j]):
            chunks.append((j, c, c * w, (c + 1) * w))

    acum = [0] * (TILES + 1)     # cumulative exp count through tile j
    for j in range(TILES):
        acum[j + 1] = acum[j] + CHN[j]

    # multi-chunk tiles accumulate into scratch columns, folded at the end
    xcol = {}
    nx = 0
    for j in range(TILES):
        if CHN[j] > 1:
            for c in range(CHN[j]):
                xcol[(j, c)] = nx
                nx += 1
    N_FOLD = sum(1 for j in range(TILES) if CHN[j] > 1)

    A_E = acum[TILES]            # all exps done
    A_EN = A_E + 1
    A_LND = A_EN + 1
    A_RES = A_LND + 1
    V_FOLD = N_FOLD              # summ folds done
    V_DEN = V_FOLD + 1
    V_LG = V_DEN + 1

    slot_chunks = [0] * NBUF
    for j in range(TILES):
        slot_chunks[j % NBUF] = max(slot_chunks[j % NBUF], CHN[j])

    nc = bass.Bass()
    logits = nc.declare_dram_parameter("logits", [ROWS, C], F32, isOutput=False)
    toff = nc.declare_dram_parameter("toff", [P, TILES], I32, isOutput=False)
    out = nc.declare_dram_parameter("out", [1, 1], F32, isOutput=True)

    logits_t = logits.rearrange("(n p) c -> n p c", p=P)
    logits_flat = logits.rearrange("r c -> (r c) ()")

    with ExitStack() as ctx:
        en_ctx = ctx.enter_context
        tb = [
            en_ctx(nc.sbuf_tensor(f"tb{i}", [P, C], F32)) for i in range(NBUF)
        ]
        g_act = en_ctx(nc.sbuf_tensor([P, C], FP8))   # unused elementwise out
        bias_m = en_ctx(nc.sbuf_tensor([P, 1], F32))
        bias_eps = en_ctx(nc.sbuf_tensor([P, 1], F32))
        off_sb = en_ctx(nc.sbuf_tensor([P, TILES], I32))
        summ = en_ctx(nc.sbuf_tensor([P, TILES], F32))
        summ_x = en_ctx(nc.sbuf_tensor([P, max(nx, 1)], F32))
        num = en_ctx(nc.sbuf_tensor([P, TILES], F32))
        en = en_ctx(nc.sbuf_tensor([P, TILES], F32))
        denom = en_ctx(nc.sbuf_tensor([P, TILES], F32))
        lnd = en_ctx(nc.sbuf_tensor([P, TILES], F32))
        lg = en_ctx(nc.sbuf_tensor([P, TILES], F32))
        partial = en_ctx(nc.sbuf_tensor([P, 1], F32))
        red = en_ctx(nc.sbuf_tensor([1, 1], F32))
        res = en_ctx(nc.sbuf_tensor([1, 1], F32))

        to_sem = en_ctx(nc.semaphore("to_sem"))
        num_sem = en_ctx(nc.semaphore("num_sem"))
        cs = [
            [en_ctx(nc.semaphore(f"cs{s}_{c}")) for c in range(slot_chunks[s])]
            for s in range(NBUF)
        ]
        out_sem = en_ctx(nc.semaphore("out_sem"))
        v_sem = en_ctx(nc.semaphore("v_sem"))
        a_sem = en_ctx(nc.semaphore("a_sem"))
        b_sem = en_ctx(nc.semaphore("b_sem"))
        p_sem = en_ctx(nc.semaphore("p_sem"))

        block = en_ctx(nc.Block())

        _thr = {}

        def chunk_wait_threshold(j, c):
            """Cumulative value of cs[j % NBUF][c] once chunk (j, c) landed."""
            key = (j, c)
            if key not in _thr:
                n = sum(1 for j2 in range(j + 1) if j2 % NBUF == j % NBUF
                        and CHN[j2] > c)
                _thr[key] = 16 * n
            return _thr[key]

        @block.sync
        def _(sync):
            for j, c, lo, hi in chunks:
                if c == 0 and j >= NBUF:
                    # slot reuse: only the scalar engine reads tiles now
                    sync.wait_ge(a_sem, acum[j - NBUF + 1])
                sync.dma_start(
                    out=tb[j % NBUF][:, lo:hi], in_=logits_t[j][:, lo:hi]
                ).then_inc(cs[j % NBUF][c], 16)
            sync.wait_ge(a_sem, A_RES)
            sync.dma_start(out=out[:], in_=res[:]).then_inc(out_sem, 16)

        @block.gpsimd
        def _(gpsimd):
            gpsimd.dma_start(out=off_sb.ap(), in_=toff[:]).then_inc(to_sem, 16)
            gpsimd.wait_ge(to_sem, 16)
            # gathering DMAs fetch every target logit straight from DRAM; the
            # hardware DGE supports one offset per partition per transfer, so
            # one [128, 1] gather per tile column
            for i in range(TILES):
                gpsimd.indirect_dma_start(
                    out=num.ap()[:, i : i + 1],
                    out_offset=None,
                    in_=logits_flat,
                    in_offset=bass.IndirectOffsetOnAxis(
                        ap=off_sb.ap()[:, i : i + 1], axis=0
                    ),
                ).then_inc(num_sem, 16)
            gpsimd.wait_ge(v_sem, V_LG)
            gpsimd.tensor_reduce(
                red[:], partial[:], axis=mybir.AxisListType.C, op=Alu.add
            ).then_inc(p_sem, 1)

        @block.vector
        def _(vector):
            vector.memset(bias_m[:], M)
            vector.memset(bias_eps[:], EPS).then_inc(b_sem, 1)
            # fold multi-chunk tiles' partial sums into their summ column
            vector.wait_ge(a_sem, A_E)
            v = 0
            for j in range(TILES):
                if CHN[j] > 1:
                    x0 = xcol[(j, 0)]
                    vector.wait_ge(v_sem, v)
                    vector.tensor_reduce(
                        summ[:, j : j + 1],
                        summ_x[:, x0 : x0 + CHN[j]],
                        axis=mybir.AxisListType.X,
                        op=Alu.add,
                    ).then_inc(v_sem, 1)
                    v += 1
            # denom = en * (1 - e^M) + summ
            vector.wait_ge(a_sem, A_EN)
            vector.wait_ge(v_sem, V_FOLD)
            vector.scalar_tensor_tensor(
                out=denom[:],
                in0=en[:],
                scalar=1.0 - EXP_M,
                in1=summ[:],
                op0=Alu.mult,
                op1=Alu.add,
            ).then_inc(v_sem, 1)
            # L = num - ln(denom + eps), accumulated per row
            vector.wait_ge(a_sem, A_LND)
            vector.wait_ge(v_sem, V_DEN)
            vector.wait_ge(num_sem, 16 * TILES)
            vector.scalar_tensor_tensor(
                out=lg[:],
                in0=num[:],
                scalar=1.0,
                in1=lnd[:],
                op0=Alu.mult,
                op1=Alu.subtract,
                accum_out=partial[:],
            ).then_inc(v_sem, 1)

        @block.scalar
        def _(scalar):
            scalar.wait_ge(b_sem, 1)
            k = 0
            for j, c, lo, hi in chunks:
                acc = (
                    summ[:, j : j + 1]
                    if CHN[j] == 1
                    else summ_x[:, xcol[(j, c)] : xcol[(j, c)] + 1]
                )
                scalar.wait_ge(a_sem, k)
                scalar.wait_ge(cs[j % NBUF][c], chunk_wait_threshold(j, c))
                scalar.activation(
                    out=g_act[:, 0 : hi - lo],
                    in_=tb[j % NBUF][:, lo:hi],
                    func=Act.Exp,
                    bias=bias_m[:],
                    scale=1.0,
                    accum_out=acc,
                ).then_inc(a_sem, 1)
                k += 1
            scalar.wait_ge(num_sem, 16 * TILES)
            scalar.activation(out=en[:], in_=num[:], func=Act.Exp).then_inc(
                a_sem, 1
            )
            scalar.wait_ge(v_sem, V_DEN)
            scalar.activation(
                out=lnd[:], in_=denom[:], func=Act.Ln, bias=bias_eps[:]
            ).then_inc(a_sem, 1)
            scalar.wait_ge(p_sem, 1)
            scalar.mul(res[:], red[:], -1.0 / N_TOTAL).then_inc(a_sem, 1)

    return nc


def _get_nc():
    if "nc" not in _CACHE:
        _CACHE["nc"] = _build_nc()
    return _CACHE["nc"]


def kernel(logits, targets):
    global LAST_RESULT
    from concourse.bass_utils import run_bass_kernel_spmd

    logits = np.ascontiguousarray(np.asarray(logits), dtype=np.float32)
    targets = np.asarray(targets).astype(np.int64)
    assert logits.shape == (N_TOTAL, C), logits.shape
    assert targets.shape == (N_TOTAL,), targets.shape

    # tile j, partition p holds shard row j*128 + p; offsets are flat element
    # indices into the core's [ROWS, C] shard for the indirect gather DMA
    rows = np.arange(TILES)[None, :] * P + np.arange(P)[:, None]   # [P, TILES]

    in_maps = []
    for k in range(N_CORES):
        lo, hi = k * ROWS, (k + 1) * ROWS
        shard = logits[lo:hi]
        tg = targets[lo:hi]
        toff = (rows * C + tg[rows]).astype(np.int32)
        in_maps.append({"logits": shard, "toff": np.ascontiguousarray(toff)})

    nc = _get_nc()
    result = run_bass_kernel_spmd(
        nc, in_maps, core_ids=list(range(N_CORES)), trace=PROFILE
    )
    LAST_RESULT = result
    total = np.float64(0.0)
    for r in result.results:
        total += np.float64(r["out"].reshape(-1)[0])
    return np.float32(total)


# revision 34
# speedup vs baseline: 1.2056x; 1.0094x over previous
"""AMS loss kernel for Trainium2, data-parallel over 8 NeuronCores.

Reference computation (per row r of logits [N, C], target t_r):
    num_r   = logits[r, t_r]
    denom_r = exp(num_r) + (sum_j exp(logits[r, j])) * e^M - exp(num_r) * e^M
    L_r     = num_r - log(denom_r + EPS)
    out     = -mean_r(L_r)

Sharding: rows (N=16384) split evenly across 8 cores (2048 rows each).
Per core:
 - The target logits num_r are fetched straight from DRAM by one
   indirect (gathering) DMA on gpsimd's software DGE, using host-packed
   flat element offsets -- no compute-engine gather pass at all.
 - The scalar engine streams the shard (16 row-tiles of [128, 10000])
   computing exp(x + M) with a fused per-row accumulate (accum_out).
 - The vector engine computes the tiny [128, 16] epilogue; gpsimd folds
   the per-row losses across partitions; the host sums 8 partial scalars.

Raw Bass (no Tile): Tile's auto-generated per-instruction waits overflow
the small sync-wait slot budgets of the fused-reduce and DMA instruction
formats, so synchronization is explicit standalone wait_ge per engine.

Schedule notes (from NTFF profiling):
 - The logits stream runs at HBM line rate (~390-412 GB/s) on the SP
   HWDGE FIFO queue with 40 KB per-partition lines; smaller lines drop
   the rate, so only the last tile is column-split (4 x 2500) to shrink
   the end-of-stream exposure to one small exp.
 - The chip power-shares between clock domains run-to-run (some runs
   have ~20% slower compute clocks, some ~15% slower HBM); with the
   gather off the vector engine, the scalar engine's exp is the only
   per-tile compute and it has slack in both regimes.
"""

import sys
import numpy as np

for _p in ("/opt/trn_rl_repo",):
    if _p not in sys.path:
        sys.path.insert(0, _p)

N_TOTAL = 16384
C = 10000
N_CORES = 8
ROWS = N_TOTAL // N_CORES        # 2048 rows per core
P = 128                          # partitions
TILES = ROWS // P                # 16 row-tiles per core
M = 0.4
EPS = 1e-10
NBUF = 4                         # row-tile buffer slots

# chunk widths per tile: the last tiles are split (tapered) so the final
# exposed exp after the DMA stream ends is small
CHW = {14: [5000, 5000], 15: [5000, 2500, 1250, 1250]}
CHN = [len(CHW.get(j, [0])) if j in CHW else 1 for j in range(TILES)]

PROFILE = False                  # set True (e.g. by test.py) to capture NTFF profile
LAST_RESULT = None               # BassKernelResults of the last run (for profiling)

_CACHE = {}


def _build_nc():
    from contextlib import ExitStack

    import concourse.bass as bass
    import concourse.mybir as mybir

    F32 = mybir.dt.float32
    FP8 = mybir.dt.float8e4
    I32 = mybir.dt.int32
    Alu = mybir.AluOpType
    Act = mybir.ActivationFunctionType

    EXP_M = float(np.exp(np.float32(M)))

    # chunk table: (tile j, chunk c, col_lo, col_hi)
    chunks = []
    for j in range(TILES):
        widths = CHW.get(j, [C])
        lo = 0
        for c, w in enumerate(widths):
            chunks.append((j, c, lo, lo + w))
            lo += w
        assert lo == C

    acum = [0] * (TILES + 1)     # cumulative exp count through tile j
    for j in range(TILES):
        acum[j + 1] = acum[j] + CHN[j]

    # multi-chunk tiles accumulate into scratch columns, folded at the end
    xcol = {}
    nx = 0
    for j in range(TILES):
        if CHN[j] > 1:
            for c in range(CHN[j]):
                xcol[(j, c)] = nx
                nx += 1
    N_FOLD = sum(1 for j in range(TILES) if CHN[j] > 1)

    A_E = acum[TILES]            # all exps done
    A_LND = A_E + 1
    A_RES = A_LND + 1
    V_FOLD = N_FOLD              # summ folds done
    V_DEN = V_FOLD + 1
    V_LG = V_DEN + 1

    slot_chunks = [0] * NBUF
    for j in range(TILES):
        slot_chunks[j % NBUF] = max(slot_chunks[j % NBUF], CHN[j])

    nc = bass.Bass()
    logits = nc.declare_dram_parameter("logits", [ROWS, C], F32, isOutput=False)
    toff = nc.declare_dram_parameter("toff", [P, TILES], I32, isOutput=False)
    out = nc.declare_dram_parameter("out", [1, 1], F32, isOutput=True)

    logits_t = logits.rearrange("(n p) c -> n p c", p=P)
    logits_flat = logits.rearrange("r c -> (r c) ()")

    with ExitStack() as ctx:
        en_ctx = ctx.enter_context
        tb = [
            en_ctx(nc.sbuf_tensor(f"tb{i}", [P, C], F32)) for i in range(NBUF)
        ]
        g_act = en_ctx(nc.sbuf_tensor([P, C], FP8))   # unused elementwise out
        bias_m = en_ctx(nc.sbuf_tensor([P, 1], F32))
        bias_eps = en_ctx(nc.sbuf_tensor([P, 1], F32))
        off_sb = en_ctx(nc.sbuf_tensor([P, TILES], I32))
        summ = en_ctx(nc.sbuf_tensor([P, TILES], F32))
        summ_x = en_ctx(nc.sbuf_tensor([P, max(nx, 1)], F32))
        num = en_ctx(nc.sbuf_tensor([P, TILES], F32))
        en = en_ctx(nc.sbuf_tensor([P, TILES], F32))
        denom = en_ctx(nc.sbuf_tensor([P, TILES], F32))
        lnd = en_ctx(nc.sbuf_tensor([P, TILES], F32))
        lg = en_ctx(nc.sbuf_tensor([P, TILES], F32))
        partial = en_ctx(nc.sbuf_tensor([P, 1], F32))
        red = en_ctx(nc.sbuf_tensor([1, 1], F32))
        res = en_ctx(nc.sbuf_tensor([1, 1], F32))

        to_sem = en_ctx(nc.semaphore("to_sem"))
        num_sem = en_ctx(nc.semaphore("num_sem"))
        cs = [
            [en_ctx(nc.semaphore(f"cs{s}_{c}")) for c in range(slot_chunks[s])]
            for s in range(NBUF)
        ]
        out_sem = en_ctx(nc.semaphore("out_sem"))
        en_sem = en_ctx(nc.semaphore("en_sem"))
        v_sem = en_ctx(nc.semaphore("v_sem"))
        a_sem = en_ctx(nc.semaphore("a_sem"))
        b_sem = en_ctx(nc.semaphore("b_sem"))
        p_sem = en_ctx(nc.semaphore("p_sem"))

        block = en_ctx(nc.Block())

        _thr = {}

        def chunk_wait_threshold(j, c):
            """Cumulative value of cs[j % NBUF][c] once chunk (j, c) landed."""
            key = (j, c)
            if key not in _thr:
                n = sum(1 for j2 in range(j + 1) if j2 % NBUF == j % NBUF
                        and CHN[j2] > c)
                _thr[key] = 16 * n
            return _thr[key]

        @block.sync
        def _(sync):
            for j, c, lo, hi in chunks:
                if c == 0 and j >= NBUF:
                    # slot reuse: only the scalar engine reads tiles now
                    sync.wait_ge(a_sem, acum[j - NBUF + 1])
                sync.dma_start(
                    out=tb[j % NBUF][:, lo:hi], in_=logits_t[j][:, lo:hi]
                ).then_inc(cs[j % NBUF][c], 16)
            sync.wait_ge(a_sem, A_RES)
            sync.dma_start(out=out[:], in_=res[:]).then_inc(out_sem, 16)

        @block.gpsimd
        def _(gpsimd):
            gpsimd.dma_start(out=off_sb.ap(), in_=toff[:]).then_inc(to_sem, 16)
            gpsimd.wait_ge(to_sem, 16)
            # gathering DMAs fetch every target logit straight from DRAM; the
            # hardware DGE supports one offset per partition per transfer, so
            # one [128, 1] gather per tile column
            for i in range(TILES):
                gpsimd.indirect_dma_start(
                    out=num.ap()[:, i : i + 1],
                    out_offset=None,
                    in_=logits_flat,
                    in_offset=bass.IndirectOffsetOnAxis(
                        ap=off_sb.ap()[:, i : i + 1], axis=0
                    ),
                ).then_inc(num_sem, 16)
            gpsimd.wait_ge(v_sem, V_LG)
            gpsimd.tensor_reduce(
                red[:], partial[:], axis=mybir.AxisListType.C, op=Alu.add
            ).then_inc(p_sem, 1)

        @block.vector
        def _(vector):
            vector.memset(bias_m[:], M)
            vector.memset(bias_eps[:], EPS).then_inc(b_sem, 1)
            # fold multi-chunk tiles' partial sums into their summ column
            vector.wait_ge(a_sem, A_E)
            v = 0
            for j in range(TILES):
                if CHN[j] > 1:
                    x0 = xcol[(j, 0)]
                    vector.wait_ge(v_sem, v)
                    vector.tensor_reduce(
                        summ[:, j : j + 1],
                        summ_x[:, x0 : x0 + CHN[j]],
                        axis=mybir.AxisListType.X,
                        op=Alu.add,
                    ).then_inc(v_sem, 1)
                    v += 1
            # denom = en * (1 - e^M) + summ
            vector.wait_ge(en_sem, 1)
            vector.wait_ge(a_sem, A_E)
            vector.wait_ge(v_sem, V_FOLD)
            vector.scalar_tensor_tensor(
                out=denom[:],
                in0=en[:],
                scalar=1.0 - EXP_M,
                in1=summ[:],
                op0=Alu.mult,
                op1=Alu.add,
            ).then_inc(v_sem, 1)
            # L = num - ln(denom + eps), accumulated per row
            vector.wait_ge(a_sem, A_LND)
            vector.wait_ge(v_sem, V_DEN)
            vector.wait_ge(num_sem, 16 * TILES)
            vector.scalar_tensor_tensor(
                out=lg[:],
                in0=num[:],
                scalar=1.0,
                in1=lnd[:],
                op0=Alu.mult,
                op1=Alu.subtract,
                accum_out=partial[:],
            ).then_inc(v_sem, 1)

        @block.scalar
        def _(scalar):
            scalar.wait_ge(b_sem, 1)
            k = 0
            for j, c, lo, hi in chunks:
                acc = (
                    summ[:, j : j + 1]
                    if CHN[j] == 1
                    else summ_x[:, xcol[(j, c)] : xcol[(j, c)] + 1]
                )
                scalar.wait_ge(a_sem, k)
                scalar.wait_ge(cs[j % NBUF][c], chunk_wait_threshold(j, c))
                scalar.activation(
                    out=g_act[:, 0 : hi - lo],
                    in_=tb[j % NBUF][:, lo:hi],
                    func=Act.Exp,
                    bias=bias_m[:],
                    scale=1.0,
                    accum_out=acc,
                ).then_inc(a_sem, 1)
                k += 1
                if k == acum[9]:
                    # en = exp(num) computed mid-stream: num is gathered by
                    # ~40us, and this keeps it off the end-of-kernel chain
                    scalar.wait_ge(num_sem, 16 * TILES)
                    scalar.activation(
                        out=en[:], in_=num[:], func=Act.Exp
                    ).then_inc(en_sem, 1)
            scalar.wait_ge(v_sem, V_DEN)
            scalar.activation(
                out=lnd[:], in_=denom[:], func=Act.Ln, bias=bias_eps[:]
            ).then_inc(a_sem, 1)
            scalar.wait_ge(p_sem, 1)
            scalar.mul(res[:], red[:], -1.0 / N_TOTAL).then_inc(a_sem, 1)

    return nc


def _get_nc():
    if "nc" not in _CACHE:
        _CACHE["nc"] = _build_nc()
    return _CACHE["nc"]


def kernel(logits, targets):
    global LAST_RESULT
    from concourse.bass_utils import run_bass_kernel_spmd

    logits = np.ascontiguousarray(np.asarray(logits), dtype=np.float32)
    targets = np.asarray(targets).astype(np.int64)
    assert logits.shape == (N_TOTAL, C), logits.shape
    assert targets.shape == (N_TOTAL,), targets.shape

    # tile j, partition p holds shard row j*128 + p; offsets are flat element
    # indices into the core's [ROWS, C] shard for the indirect gather DMA
    rows = np.arange(TILES)[None, :] * P + np.arange(P)[:, None]   # [P, TILES]

    in_maps = []
    for k in range(N_CORES):
        lo, hi = k * ROWS, (k + 1) * ROWS
        shard = logits[lo:hi]
        tg = targets[lo:hi]
        toff = (rows * C + tg[rows]).astype(np.int32)
        in_maps.append({"logits": shard, "toff": np.ascontiguousarray(toff)})

    nc = _get_nc()
    result = run_bass_kernel_spmd(
        nc, in_maps, core_ids=list(range(N_CORES)), trace=PROFILE
    )
    LAST_RESULT = result
    total = np.float64(0.0)
    for r in result.results:
        total += np.float64(r["out"].reshape(-1)[0])
    return np.float32(total)


# revision 35
# speedup vs baseline: 1.2100x; 1.0037x over previous
"""AMS loss kernel for Trainium2, data-parallel over 8 NeuronCores.

Reference computation (per row r of logits [N, C], target t_r):
    num_r   = logits[r, t_r]
    denom_r = exp(num_r) + (sum_j exp(logits[r, j])) * e^M - exp(num_r) * e^M
    L_r     = num_r - log(denom_r + EPS)
    out     = -mean_r(L_r)

Sharding: rows (N=16384) split evenly across 8 cores (2048 rows each).
Per core:
 - The target logits num_r are fetched straight from DRAM by one
   indirect (gathering) DMA on gpsimd's software DGE, using host-packed
   flat element offsets -- no compute-engine gather pass at all.
 - The scalar engine streams the shard (16 row-tiles of [128, 10000])
   computing exp(x + M) with a fused per-row accumulate (accum_out).
 - The vector engine computes the tiny [128, 16] epilogue; gpsimd folds
   the per-row losses across partitions; the host sums 8 partial scalars.

Raw Bass (no Tile): Tile's auto-generated per-instruction waits overflow
the small sync-wait slot budgets of the fused-reduce and DMA instruction
formats, so synchronization is explicit standalone wait_ge per engine.

Schedule notes (from NTFF profiling):
 - The logits stream runs at HBM line rate (~390-412 GB/s) on the SP
   HWDGE FIFO queue with 40 KB per-partition lines; smaller lines drop
   the rate, so only the last tile is column-split (4 x 2500) to shrink
   the end-of-stream exposure to one small exp.
 - The chip power-shares between clock domains run-to-run (some runs
   have ~20% slower compute clocks, some ~15% slower HBM); with the
   gather off the vector engine, the scalar engine's exp is the only
   per-tile compute and it has slack in both regimes.
"""

import sys
import numpy as np

for _p in ("/opt/trn_rl_repo",):
    if _p not in sys.path:
        sys.path.insert(0, _p)

N_TOTAL = 16384
C = 10000
N_CORES = 8
ROWS = N_TOTAL // N_CORES        # 2048 rows per core
P = 128                          # partitions
TILES = ROWS // P                # 16 row-tiles per core
M = 0.4
EPS = 1e-10
NBUF = 4                         # row-tile buffer slots

# chunk widths per tile: the last tiles are split (tapered) so the final
# exposed exp after the DMA stream ends is small
CHW = {12: [5000, 5000], 13: [5000, 5000], 14: [5000, 5000],
       15: [5000, 2500, 1250, 1250]}
CHN = [len(CHW.get(j, [0])) if j in CHW else 1 for j in range(TILES)]

PROFILE = False                  # set True (e.g. by test.py) to capture NTFF profile
LAST_RESULT = None               # BassKernelResults of the last run (for profiling)

_CACHE = {}


def _build_nc():
    from contextlib import ExitStack

    import concourse.bass as bass
    import concourse.mybir as mybir

    F32 = mybir.dt.float32
    FP8 = mybir.dt.float8e4
    I32 = mybir.dt.int32
    Alu = mybir.AluOpType
    Act = mybir.ActivationFunctionType

    EXP_M = float(np.exp(np.float32(M)))

    # chunk table: (tile j, chunk c, col_lo, col_hi)
    chunks = []
    for j in range(TILES):
        widths = CHW.get(j, [C])
        lo = 0
        for c, w in enumerate(widths):
            chunks.append((j, c, lo, lo + w))
            lo += w
        assert lo == C

    acum = [0] * (TILES + 1)     # cumulative exp count through tile j
    for j in range(TILES):
        acum[j + 1] = acum[j] + CHN[j]

    # multi-chunk tiles accumulate into scratch columns, folded at the end
    xcol = {}
    nx = 0
    for j in range(TILES):
        if CHN[j] > 1:
            for c in range(CHN[j]):
                xcol[(j, c)] = nx
                nx += 1
    N_FOLD = sum(1 for j in range(TILES) if CHN[j] > 1)

    A_E = acum[TILES]            # all exps done
    A_LND = A_E + 1
    A_RES = A_LND + 1
    V_FOLD = N_FOLD              # summ folds done
    V_DEN = V_FOLD + 1
    V_LG = V_DEN + 1

    slot_chunks = [0] * NBUF
    for j in range(TILES):
        slot_chunks[j % NBUF] = max(slot_chunks[j % NBUF], CHN[j])

    nc = bass.Bass()
    logits = nc.declare_dram_parameter("logits", [ROWS, C], F32, isOutput=False)
    toff = nc.declare_dram_parameter("toff", [P, TILES], I32, isOutput=False)
    out = nc.declare_dram_parameter("out", [1, 1], F32, isOutput=True)

    logits_t = logits.rearrange("(n p) c -> n p c", p=P)
    logits_flat = logits.rearrange("r c -> (r c) ()")

    with ExitStack() as ctx:
        en_ctx = ctx.enter_context
        tb = [
            en_ctx(nc.sbuf_tensor(f"tb{i}", [P, C], F32)) for i in range(NBUF)
        ]
        g_act = en_ctx(nc.sbuf_tensor([P, C], FP8))   # unused elementwise out
        bias_m = en_ctx(nc.sbuf_tensor([P, 1], F32))
        bias_eps = en_ctx(nc.sbuf_tensor([P, 1], F32))
        off_sb = en_ctx(nc.sbuf_tensor([P, TILES], I32))
        summ = en_ctx(nc.sbuf_tensor([P, TILES], F32))
        summ_x = en_ctx(nc.sbuf_tensor([P, max(nx, 1)], F32))
        num = en_ctx(nc.sbuf_tensor([P, TILES], F32))
        en = en_ctx(nc.sbuf_tensor([P, TILES], F32))
        denom = en_ctx(nc.sbuf_tensor([P, TILES], F32))
        lnd = en_ctx(nc.sbuf_tensor([P, TILES], F32))
        lg = en_ctx(nc.sbuf_tensor([P, TILES], F32))
        partial = en_ctx(nc.sbuf_tensor([P, 1], F32))
        red = en_ctx(nc.sbuf_tensor([1, 1], F32))
        res = en_ctx(nc.sbuf_tensor([1, 1], F32))

        to_sem = en_ctx(nc.semaphore("to_sem"))
        num_sem = en_ctx(nc.semaphore("num_sem"))
        cs = [
            [en_ctx(nc.semaphore(f"cs{s}_{c}")) for c in range(slot_chunks[s])]
            for s in range(NBUF)
        ]
        out_sem = en_ctx(nc.semaphore("out_sem"))
        en_sem = en_ctx(nc.semaphore("en_sem"))
        v_sem = en_ctx(nc.semaphore("v_sem"))
        a_sem = en_ctx(nc.semaphore("a_sem"))
        b_sem = en_ctx(nc.semaphore("b_sem"))
        p_sem = en_ctx(nc.semaphore("p_sem"))

        block = en_ctx(nc.Block())

        _thr = {}

        def chunk_wait_threshold(j, c):
            """Cumulative value of cs[j % NBUF][c] once chunk (j, c) landed."""
            key = (j, c)
            if key not in _thr:
                n = sum(1 for j2 in range(j + 1) if j2 % NBUF == j % NBUF
                        and CHN[j2] > c)
                _thr[key] = 16 * n
            return _thr[key]

        @block.sync
        def _(sync):
            for j, c, lo, hi in chunks:
                if c == 0 and j >= NBUF:
                    # slot reuse: only the scalar engine reads tiles now
                    sync.wait_ge(a_sem, acum[j - NBUF + 1])
                sync.dma_start(
                    out=tb[j % NBUF][:, lo:hi], in_=logits_t[j][:, lo:hi]
                ).then_inc(cs[j % NBUF][c], 16)
            sync.wait_ge(a_sem, A_RES)
            sync.dma_start(out=out[:], in_=res[:]).then_inc(out_sem, 16)

        @block.gpsimd
        def _(gpsimd):
            gpsimd.dma_start(out=off_sb.ap(), in_=toff[:]).then_inc(to_sem, 16)
            gpsimd.wait_ge(to_sem, 16)
            # gathering DMAs fetch every target logit straight from DRAM; the
            # hardware DGE supports one offset per partition per transfer, so
            # one [128, 1] gather per tile column
            for i in range(TILES):
                gpsimd.indirect_dma_start(
                    out=num.ap()[:, i : i + 1],
                    out_offset=None,
                    in_=logits_flat,
                    in_offset=bass.IndirectOffsetOnAxis(
                        ap=off_sb.ap()[:, i : i + 1], axis=0
                    ),
                ).then_inc(num_sem, 16)
            gpsimd.wait_ge(v_sem, V_LG)
            gpsimd.tensor_reduce(
                red[:], partial[:], axis=mybir.AxisListType.C, op=Alu.add
            ).then_inc(p_sem, 1)

        @block.vector
        def _(vector):
            vector.memset(bias_m[:], M)
            vector.memset(bias_eps[:], EPS).then_inc(b_sem, 1)
            # fold multi-chunk tiles' partial sums into their summ column,
            # progressively as each tile's exps finish
            v = 0
            for j in range(TILES):
                if CHN[j] > 1:
                    x0 = xcol[(j, 0)]
                    vector.wait_ge(a_sem, acum[j + 1])
                    vector.wait_ge(v_sem, v)
                    vector.tensor_reduce(
                        summ[:, j : j + 1],
                        summ_x[:, x0 : x0 + CHN[j]],
                        axis=mybir.AxisListType.X,
                        op=Alu.add,
                    ).then_inc(v_sem, 1)
                    v += 1
            # denom = en * (1 - e^M) + summ
            vector.wait_ge(en_sem, 1)
            vector.wait_ge(a_sem, A_E)
            vector.wait_ge(v_sem, V_FOLD)
            vector.scalar_tensor_tensor(
                out=denom[:],
                in0=en[:],
                scalar=1.0 - EXP_M,
                in1=summ[:],
                op0=Alu.mult,
                op1=Alu.add,
            ).then_inc(v_sem, 1)
            # L = num - ln(denom + eps), accumulated per row
            vector.wait_ge(a_sem, A_LND)
            vector.wait_ge(v_sem, V_DEN)
            vector.wait_ge(num_sem, 16 * TILES)
            vector.scalar_tensor_tensor(
                out=lg[:],
                in0=num[:],
                scalar=1.0,
                in1=lnd[:],
                op0=Alu.mult,
                op1=Alu.subtract,
                accum_out=partial[:],
            ).then_inc(v_sem, 1)

        @block.scalar
        def _(scalar):
            scalar.wait_ge(b_sem, 1)
            k = 0
            for j, c, lo, hi in chunks:
                acc = (
                    summ[:, j : j + 1]
                    if CHN[j] == 1
                    else summ_x[:, xcol[(j, c)] : xcol[(j, c)] + 1]
                )
                scalar.wait_ge(a_sem, k)
                scalar.wait_ge(cs[j % NBUF][c], chunk_wait_threshold(j, c))
                scalar.activation(
                    out=g_act[:, 0 : hi - lo],
                    in_=tb[j % NBUF][:, lo:hi],
                    func=Act.Exp,
                    bias=bias_m[:],
                    scale=1.0,
                    accum_out=acc,
                ).then_inc(a_sem, 1)
                k += 1
                if k == acum[9]:
                    # en = exp(num) computed mid-stream: num is gathered by
                    # ~40us, and this keeps it off the end-of-kernel chain
                    scalar.wait_ge(num_sem, 16 * TILES)
                    scalar.activation(
                        out=en[:], in_=num[:], func=Act.Exp
                    ).then_inc(en_sem, 1)
            scalar.wait_ge(v_sem, V_DEN)
            scalar.activation(
                out=lnd[:], in_=denom[:], func=Act.Ln, bias=bias_eps[:]
            ).then_inc(a_sem, 1)
            scalar.wait_ge(p_sem, 1)
            scalar.mul(res[:], red[:], -1.0 / N_TOTAL).then_inc(a_sem, 1)

    return nc


def _get_nc():
    if "nc" not in _CACHE:
        _CACHE["nc"] = _build_nc()
    return _CACHE["nc"]


def kernel(logits, targets):
    global LAST_RESULT
    from concourse.bass_utils import run_bass_kernel_spmd

    logits = np.ascontiguousarray(np.asarray(logits), dtype=np.float32)
    targets = np.asarray(targets).astype(np.int64)
    assert logits.shape == (N_TOTAL, C), logits.shape
    assert targets.shape == (N_TOTAL,), targets.shape

    # tile j, partition p holds shard row j*128 + p; offsets are flat element
    # indices into the core's [ROWS, C] shard for the indirect gather DMA
    rows = np.arange(TILES)[None, :] * P + np.arange(P)[:, None]   # [P, TILES]

    in_maps = []
    for k in range(N_CORES):
        lo, hi = k * ROWS, (k + 1) * ROWS
        shard = logits[lo:hi]
        tg = targets[lo:hi]
        toff = (rows * C + tg[rows]).astype(np.int32)
        in_maps.append({"logits": shard, "toff": np.ascontiguousarray(toff)})

    nc = _get_nc()
    result = run_bass_kernel_spmd(
        nc, in_maps, core_ids=list(range(N_CORES)), trace=PROFILE
    )
    LAST_RESULT = result
    total = np.float64(0.0)
    for r in result.results:
        total += np.float64(r["out"].reshape(-1)[0])
    return np.float32(total)


# revision 36
# speedup vs baseline: 1.2172x; 1.0060x over previous
"""AMS loss kernel for Trainium2, data-parallel over 8 NeuronCores.

Reference computation (per row r of logits [N, C], target t_r):
    num_r   = logits[r, t_r]
    denom_r = exp(num_r) + (sum_j exp(logits[r, j])) * e^M - exp(num_r) * e^M
    L_r     = num_r - log(denom_r + EPS)
    out     = -mean_r(L_r)

Sharding: rows (N=16384) split evenly across 8 cores (2048 rows each).
Per core:
 - The target logits num_r are fetched straight from DRAM by one
   indirect (gathering) DMA on gpsimd's software DGE, using host-packed
   flat element offsets -- no compute-engine gather pass at all.
 - The scalar engine streams the shard (16 row-tiles of [128, 10000])
   computing exp(x + M) with a fused per-row accumulate (accum_out).
 - The vector engine computes the tiny [128, 16] epilogue; gpsimd folds
   the per-row losses across partitions; the host sums 8 partial scalars.

Raw Bass (no Tile): Tile's auto-generated per-instruction waits overflow
the small sync-wait slot budgets of the fused-reduce and DMA instruction
formats, so synchronization is explicit standalone wait_ge per engine.

Schedule notes (from NTFF profiling):
 - The logits stream runs at HBM line rate (~390-412 GB/s) on the SP
   HWDGE FIFO queue with 40 KB per-partition lines; smaller lines drop
   the rate, so only the last tile is column-split (4 x 2500) to shrink
   the end-of-stream exposure to one small exp.
 - The chip power-shares between clock domains run-to-run (some runs
   have ~20% slower compute clocks, some ~15% slower HBM); with the
   gather off the vector engine, the scalar engine's exp is the only
   per-tile compute and it has slack in both regimes.
"""

import sys
import numpy as np

for _p in ("/opt/trn_rl_repo",):
    if _p not in sys.path:
        sys.path.insert(0, _p)

N_TOTAL = 16384
C = 10000
N_CORES = 8
ROWS = N_TOTAL // N_CORES        # 2048 rows per core
P = 128                          # partitions
TILES = ROWS // P                # 16 row-tiles per core
M = 0.4
EPS = 1e-10
NBUF = 4                         # row-tile buffer slots

# chunk widths per tile: the last tiles are split (tapered) so the final
# exposed exp after the DMA stream ends is small
CHW = {12: [5000, 5000], 13: [5000, 5000], 14: [2500, 2500, 2500, 2500],
       15: [2500, 2500, 2500, 1250, 1250]}
CHN = [len(CHW.get(j, [0])) if j in CHW else 1 for j in range(TILES)]

PROFILE = False                  # set True (e.g. by test.py) to capture NTFF profile
LAST_RESULT = None               # BassKernelResults of the last run (for profiling)

_CACHE = {}


def _build_nc():
    from contextlib import ExitStack

    import concourse.bass as bass
    import concourse.mybir as mybir

    F32 = mybir.dt.float32
    FP8 = mybir.dt.float8e4
    I32 = mybir.dt.int32
    Alu = mybir.AluOpType
    Act = mybir.ActivationFunctionType

    EXP_M = float(np.exp(np.float32(M)))

    # chunk table: (tile j, chunk c, col_lo, col_hi)
    chunks = []
    for j in range(TILES):
        widths = CHW.get(j, [C])
        lo = 0
        for c, w in enumerate(widths):
            chunks.append((j, c, lo, lo + w))
            lo += w
        assert lo == C

    acum = [0] * (TILES + 1)     # cumulative exp count through tile j
    for j in range(TILES):
        acum[j + 1] = acum[j] + CHN[j]

    # multi-chunk tiles accumulate into scratch columns, folded at the end
    xcol = {}
    nx = 0
    for j in range(TILES):
        if CHN[j] > 1:
            for c in range(CHN[j]):
                xcol[(j, c)] = nx
                nx += 1
    N_FOLD = sum(1 for j in range(TILES) if CHN[j] > 1)

    A_E = acum[TILES]            # all exps done
    A_LND = A_E + 1
    V_FOLD = N_FOLD              # summ folds done
    V_DEN = V_FOLD + 1
    V_LG = V_DEN + 1

    slot_chunks = [0] * NBUF
    for j in range(TILES):
        slot_chunks[j % NBUF] = max(slot_chunks[j % NBUF], CHN[j])

    nc = bass.Bass()
    logits = nc.declare_dram_parameter("logits", [ROWS, C], F32, isOutput=False)
    toff = nc.declare_dram_parameter("toff", [P, TILES], I32, isOutput=False)
    out = nc.declare_dram_parameter("out", [1, 1], F32, isOutput=True)

    logits_t = logits.rearrange("(n p) c -> n p c", p=P)
    logits_flat = logits.rearrange("r c -> (r c) ()")

    with ExitStack() as ctx:
        en_ctx = ctx.enter_context
        tb = [
            en_ctx(nc.sbuf_tensor(f"tb{i}", [P, C], F32)) for i in range(NBUF)
        ]
        g_act = en_ctx(nc.sbuf_tensor([P, C], FP8))   # unused elementwise out
        bias_m = en_ctx(nc.sbuf_tensor([P, 1], F32))
        bias_eps = en_ctx(nc.sbuf_tensor([P, 1], F32))
        off_sb = en_ctx(nc.sbuf_tensor([P, TILES], I32))
        summ = en_ctx(nc.sbuf_tensor([P, TILES], F32))
        summ_x = en_ctx(nc.sbuf_tensor([P, max(nx, 1)], F32))
        num = en_ctx(nc.sbuf_tensor([P, TILES], F32))
        en = en_ctx(nc.sbuf_tensor([P, TILES], F32))
        denom = en_ctx(nc.sbuf_tensor([P, TILES], F32))
        lnd = en_ctx(nc.sbuf_tensor([P, TILES], F32))
        lg = en_ctx(nc.sbuf_tensor([P, TILES], F32))
        partial = en_ctx(nc.sbuf_tensor([P, 1], F32))
        red = en_ctx(nc.sbuf_tensor([1, 1], F32))

        to_sem = en_ctx(nc.semaphore("to_sem"))
        num_sem = en_ctx(nc.semaphore("num_sem"))
        cs = [
            [en_ctx(nc.semaphore(f"cs{s}_{c}")) for c in range(slot_chunks[s])]
            for s in range(NBUF)
        ]
        out_sem = en_ctx(nc.semaphore("out_sem"))
        en_sem = en_ctx(nc.semaphore("en_sem"))
        v_sem = en_ctx(nc.semaphore("v_sem"))
        a_sem = en_ctx(nc.semaphore("a_sem"))
        b_sem = en_ctx(nc.semaphore("b_sem"))
        p_sem = en_ctx(nc.semaphore("p_sem"))

        block = en_ctx(nc.Block())

        _thr = {}

        def chunk_wait_threshold(j, c):
            """Cumulative value of cs[j % NBUF][c] once chunk (j, c) landed."""
            key = (j, c)
            if key not in _thr:
                n = sum(1 for j2 in range(j + 1) if j2 % NBUF == j % NBUF
                        and CHN[j2] > c)
                _thr[key] = 16 * n
            return _thr[key]

        @block.sync
        def _(sync):
            for j, c, lo, hi in chunks:
                if c == 0 and j >= NBUF:
                    # slot reuse: only the scalar engine reads tiles now
                    sync.wait_ge(a_sem, acum[j - NBUF + 1])
                sync.dma_start(
                    out=tb[j % NBUF][:, lo:hi], in_=logits_t[j][:, lo:hi]
                ).then_inc(cs[j % NBUF][c], 16)
            sync.wait_ge(p_sem, 1)
            sync.dma_start(out=out[:], in_=red[:]).then_inc(out_sem, 16)

        @block.gpsimd
        def _(gpsimd):
            gpsimd.dma_start(out=off_sb.ap(), in_=toff[:]).then_inc(to_sem, 16)
            gpsimd.wait_ge(to_sem, 16)
            # gathering DMAs fetch every target logit straight from DRAM; the
            # hardware DGE supports one offset per partition per transfer, so
            # one [128, 1] gather per tile column
            for i in range(TILES):
                gpsimd.indirect_dma_start(
                    out=num.ap()[:, i : i + 1],
                    out_offset=None,
                    in_=logits_flat,
                    in_offset=bass.IndirectOffsetOnAxis(
                        ap=off_sb.ap()[:, i : i + 1], axis=0
                    ),
                ).then_inc(num_sem, 16)
            gpsimd.wait_ge(v_sem, V_LG)
            gpsimd.tensor_reduce(
                red[:], partial[:], axis=mybir.AxisListType.C, op=Alu.add
            ).then_inc(p_sem, 1)

        @block.vector
        def _(vector):
            vector.memset(bias_m[:], M)
            vector.memset(bias_eps[:], EPS).then_inc(b_sem, 1)
            # fold multi-chunk tiles' partial sums into their summ column,
            # progressively as each tile's exps finish
            v = 0
            for j in range(TILES):
                if CHN[j] > 1:
                    x0 = xcol[(j, 0)]
                    vector.wait_ge(a_sem, acum[j + 1])
                    vector.wait_ge(v_sem, v)
                    vector.tensor_reduce(
                        summ[:, j : j + 1],
                        summ_x[:, x0 : x0 + CHN[j]],
                        axis=mybir.AxisListType.X,
                        op=Alu.add,
                    ).then_inc(v_sem, 1)
                    v += 1
            # denom = en * (1 - e^M) + summ
            vector.wait_ge(en_sem, 1)
            vector.wait_ge(a_sem, A_E)
            vector.wait_ge(v_sem, V_FOLD)
            vector.scalar_tensor_tensor(
                out=denom[:],
                in0=en[:],
                scalar=1.0 - EXP_M,
                in1=summ[:],
                op0=Alu.mult,
                op1=Alu.add,
            ).then_inc(v_sem, 1)
            # L = num - ln(denom + eps), accumulated per row
            vector.wait_ge(a_sem, A_LND)
            vector.wait_ge(v_sem, V_DEN)
            vector.wait_ge(num_sem, 16 * TILES)
            vector.scalar_tensor_tensor(
                out=lg[:],
                in0=num[:],
                scalar=1.0,
                in1=lnd[:],
                op0=Alu.mult,
                op1=Alu.subtract,
                accum_out=partial[:],
            ).then_inc(v_sem, 1)

        @block.scalar
        def _(scalar):
            scalar.wait_ge(b_sem, 1)
            k = 0
            for j, c, lo, hi in chunks:
                acc = (
                    summ[:, j : j + 1]
                    if CHN[j] == 1
                    else summ_x[:, xcol[(j, c)] : xcol[(j, c)] + 1]
                )
                scalar.wait_ge(a_sem, k)
                scalar.wait_ge(cs[j % NBUF][c], chunk_wait_threshold(j, c))
                scalar.activation(
                    out=g_act[:, 0 : hi - lo],
                    in_=tb[j % NBUF][:, lo:hi],
                    func=Act.Exp,
                    bias=bias_m[:],
                    scale=1.0,
                    accum_out=acc,
                ).then_inc(a_sem, 1)
                k += 1
                if k == acum[9]:
                    # en = exp(num) computed mid-stream: num is gathered by
                    # ~40us, and this keeps it off the end-of-kernel chain
                    scalar.wait_ge(num_sem, 16 * TILES)
                    scalar.activation(
                        out=en[:], in_=num[:], func=Act.Exp
                    ).then_inc(en_sem, 1)
            scalar.wait_ge(v_sem, V_DEN)
            scalar.activation(
                out=lnd[:], in_=denom[:], func=Act.Ln, bias=bias_eps[:]
            ).then_inc(a_sem, 1)

    return nc


def _get_nc():
    if "nc" not in _CACHE:
        _CACHE["nc"] = _build_nc()
    return _CACHE["nc"]


def kernel(logits, targets):
    global LAST_RESULT
    from concourse.bass_utils import run_bass_kernel_spmd

    logits = np.ascontiguousarray(np.asarray(logits), dtype=np.float32)
    targets = np.asarray(targets).astype(np.int64)
    assert logits.shape == (N_TOTAL, C), logits.shape
    assert targets.shape == (N_TOTAL,), targets.shape

    # tile j, partition p holds shard row j*128 + p; offsets are flat element
    # indices into the core's [ROWS, C] shard for the indirect gather DMA
    rows = np.arange(TILES)[None, :] * P + np.arange(P)[:, None]   # [P, TILES]

    in_maps = []
    for k in range(N_CORES):
        lo, hi = k * ROWS, (k + 1) * ROWS
        shard = logits[lo:hi]
        tg = targets[lo:hi]
        toff = (rows * C + tg[rows]).astype(np.int32)
        in_maps.append({"logits": shard, "toff": np.ascontiguousarray(toff)})

    nc = _get_nc()
    result = run_bass_kernel_spmd(
        nc, in_maps, core_ids=list(range(N_CORES)), trace=PROFILE
    )
    LAST_RESULT = result
    total = np.float64(0.0)
    for r in result.results:
        total += np.float64(r["out"].reshape(-1)[0])
    return np.float32(-total / N_TOTAL)
